# revision 7
# baseline (speedup 1.0000x reference)
"""Multi-head attention block (B=2, N=2048, C=2560, H=40, D=64) on 8 TRN2 NeuronCores.

Sharding: tensor-parallel over heads - core c owns heads 5c..5c+4 for both
batch elements. Each core computes qkv for its heads (full token range),
attention, and its partial contribution to the output projection; the host
sums the 8 partials and adds the output bias (v-bias folded through w_proj).

Performance history: 974us (fp32r baseline) -> 734us. The kernel is within
~5% of the two-engine floor: PE busy ~690us of the span, and in phase A the
Activation engine (softmax exp, ~189us/batch) nearly matches PE (~200us/batch),
so further PE savings would hit the exp floor. fp8 DoubleRow was analyzed and
rejected: e4m3's 3-bit mantissa gives ~3-5% relative error on S/O (dot-product
relative error does not average down), vs the 2e-2 gate.

Key design points:
  - x / w_qkv / w_proj all fp16 on device (same 1 cycle/row PE rate as fp32r,
    half the DMA + SBUF). Weights loaded ONCE, per-K-tile, behind the first
    x chunk; x is host-packed [128, chunk, K, tok] so each chunk is 128
    contiguous 10KB DMA lines (the DMA engines are descriptor-rate bound:
    512B lines cost ~3x the bandwidth-ideal time).
  - ~2us of dummy matmuls at t=0 trip the HAM activity window so phase-Q
    starts at 2.4GHz.
  - V tiles padded to 128 weight columns per head: O matmuls get fast-weight
    -load (-80ns each). Even heads: dims at cols 0:64, softmax-denominator
    ones column at col 64 (denominator -> psum partition 64). Odd heads:
    dims at cols 64:128, ones at col 0 (denominator -> partition 0), so the
    odd head's output lands at partitions 64:128 and the normalize needs no
    psum partition-shift. One ACT ln instruction over partitions 0:65 covers
    both denominator rows (cost is free-size only; ln of the garbage rows
    produces unused NaNs).
  - Softmax 1/denominator = ACT exp(-ln d): both funcs live in act table 6,
    loaded manually once (the insert pass would otherwise thrash tables:
    greedy first-match sends Exp to table 0). The old DVE reciprocal was an
    iterative 8-cyc/elem op - 3.3us per row on the critical path.
  - Normalize chain per unit: copy oAB psum->SBUF scratch (frees the bank),
    ln/exp on ACT, PE ones-broadcast (M=128, FWL), then DVE multiplies
    reading scratch(SBUF) x broadcast(PSUM). The two broadcasts flush at
    k2==3 / k2==4 of the NEXT unit (bcb bank WAR + gives the ACT chain time);
    the last unit of a batch flushes inside the next phase (phQ of b+1, or
    interleaved with the drain prefixes).
  - Lone 5th head: two k-tiles share one [128,1024] S tile and one fused exp.
  - Output projection interleaved into attention k-loops (skipping the first
    slots after a q-block boundary while OT settles); y written fp16 as
    [128, 2560] row strips (one 5KB-line DMA per token tile, alternating the
    sync / gpsimd DMA rings) and summed across cores on the host.
  - Final drain pipelines each group's two OT[0]/OT[1] matmuls ahead of the
    last normalize flush on 3 rotating banks (bcb reserved: reusing it for a
    prefix would deadlock against the flush broadcast).

Hardware constraints baked in (discovered empirically):
  - matmul start=True clears has_written for the WHOLE psum bank: one
    accumulation group per bank.
  - DVE/ACT cannot shift partitions psum->sbuf, but DVE CAN shift
    partitions when writing PSUM.
  - K<128 matmuls need explicit tile_position or they run ~7x slow.
  - FWL (fast weight load) needs exactly 128 weight columns.
  - tensor_tensor on DVE may read one PSUM + one SBUF operand.
  - DVE cost is free-size x cycle, partition-count independent; reciprocal
    is 8 cycles/element.
  - Engine clocks vary run-to-run (P0 power state): ~20% of runs execute
    at PE 2.0GHz instead of 2.4GHz, inflating everything uniformly.
"""
import numpy as np

import concourse.bacc as bacc
import concourse.mybir as mybir
import concourse.tile as tile
from concourse.bass_utils import run_bass_kernel_spmd

F32 = mybir.dt.float32
F16 = mybir.dt.float16
AF = mybir.ActivationFunctionType

B, N, C = 2, 2048, 2560
H, D = 40, 64
NCORES = 8
HPC = H // NCORES            # 5 heads per core
SCALE = D ** -0.5
TOK = B * N                  # 4096
CK = 256                     # token chunk in phase Q
NCHUNK = N // CK             # 8 chunks per batch
KT16 = N // 128              # 16 k-tiles per batch
QB = 512                     # q-block
NQB = N // QB                # 4 q-blocks
KC = C // 128                # 20 contraction tiles
ACT_TABLE_LN_EXP = 6         # natural_log_exp_and_others

_CACHE = {}


def _build():
    nc = bacc.Bacc("TRN2", target_bir_lowering=False, debug=False, num_devices=NCORES)
    xH_d = nc.dram_tensor("xH", [128, TOK // CK, KC, CK], F16, kind="ExternalInput")
    wall_d = nc.dram_tensor("wall", [C, 960], F16, kind="ExternalInput")   # q|k (640) + v (320)
    wp_d = nc.dram_tensor("wp", [384, C], F16, kind="ExternalInput")       # padded 320->384
    bias_d = nc.dram_tensor("bias", [128, 6], F32, kind="ExternalInput")   # per-ft qk bias
    y_d = nc.dram_tensor("y", [TOK, C], F16, kind="ExternalOutput")

    with tile.TileContext(nc) as tc:
        with (
            tc.tile_pool(name="sb", bufs=1) as pool,
            tc.tile_pool(name="ps", bufs=1, space="PSUM") as ps,
        ):
            # ln+exp share act table 6; one manual load, the insert pass
            # then sees every Exp/Ln served and adds nothing.
            nc.scalar.add_instruction(mybir.InstLoadActFuncSet(
                name=nc.get_next_instruction_name(),
                act_func_set_id=ACT_TABLE_LN_EXP, ins=[], outs=[]))

            ones128 = pool.tile([128, 128], F16, name="ones128")
            nc.vector.memset(ones128[:], 1.0)
            bias_sb = pool.tile([128, 6], F32, name="bias_sb")
            nc.sync.dma_start(out=bias_sb[:], in_=bias_d[:])

            # ~5us of dummy matmuls while the first DMAs land: trips the HAM
            # activity window so phase-Q matmuls start at 2.4GHz.
            warm = ps.tile([128, 512], F32, tag="ypb", name="warm")
            for i in range(16):
                nc.tensor.matmul(warm[:, 0:128], ones128[:, :], ones128[:, :],
                                 start=(i == 0), stop=(i == 15))

            # V tiles: [128 tok, HPC heads x 128 cols]. Even heads: dims at
            # 0:64, ones col 64. Odd heads: dims at 64:128, ones col 0.
            # Zero + ones init is interleaved into batch-0 chunks (a big
            # upfront DVE memset burst delayed chunk-0 bias adds).
            V = [pool.tile([128, HPC * 128], F16, tag=f"v{i}", name=f"V{i}")
                 for i in range(KT16)]
            V3 = [t.rearrange("p (h e) -> p h e", h=HPC) for t in V]

            wall = pool.tile([128, KC, 960], F16, tag="w", name="wall")
            wall_r = wall_d.rearrange("(t p) f -> p t f", p=128)
            wp = pool.tile([128, 3, C], F16, tag="wpt", name="wp")

            OT = [pool.tile([128, N], F16, tag=f"ot{i}", name=f"OT{i}") for i in range(3)]
            nc.vector.memset(OT[2][64:128, :], 0.0)

            proj_q = []
            yrow_state = {}

            def yrow_put(t, n, boffq, yp):
                """CAST the psum slice into a [128, C] row strip; one 5KB-line
                DMA per t-tile (per-slice 1KB lines were descriptor-bound)."""
                key = (boffq, t)
                if key not in yrow_state:
                    yrow_state[key] = [pool.tile([128, C], F16, tag="y",
                                                 name=f"yrow{boffq}_{t}", bufs=2), 0]
                y_row, cnt = yrow_state[key]
                with nc.allow_low_precision(reason="fp16 y"):
                    nc.vector.tensor_copy(y_row[:, n * 512:(n + 1) * 512], yp[:, 0:512])
                yrow_state[key][1] = cnt + 1
                if cnt + 1 == 5:
                    # alternate rings: gpsimd (Pool) is otherwise idle, and two
                    # rings keep consecutive strip writes concurrent (the tail
                    # strips were serializing on the sync ring)
                    eng = nc.gpsimd if t % 2 else nc.sync
                    eng.dma_start(
                        out=y_d[boffq + t * 128: boffq + (t + 1) * 128, :], in_=y_row[:])
                    del yrow_state[key]

            def emit_proj():
                if not proj_q:
                    return
                t, n, OTq, boffq = proj_q.pop(0)
                tsl = slice(t * 128, (t + 1) * 128)
                nsl = slice(n * 512, (n + 1) * 512)
                yp = ps.tile([128, 512], F32, tag="ypb", name=f"yp{boffq}_{t}_{n}")
                nc.tensor.matmul(yp[:, 0:512], OTq[0][:, tsl], wp[:, 0, nsl],
                                 start=True, stop=False)
                nc.tensor.matmul(yp[:, 0:512], OTq[1][:, tsl], wp[:, 1, nsl],
                                 start=False, stop=False)
                nc.tensor.matmul(yp[:, 0:512], OTq[2][:, tsl], wp[:, 2, nsl],
                                 start=False, stop=True)
                yrow_put(t, n, boffq, yp)

            pending = [None, None]

            def flush_pending(i):
                if pending[i] is not None:
                    pending[i]()
                    pending[i] = None

            for b in range(B):
                boff = b * N

                # ---------------- phase Q: qkv projections ----------------
                scope_q = nc.enter_named_scope(f"phQ{b}", False)
                QT = [pool.tile([128, N], F16, tag=f"qt{i}", name=f"QT{i}_{b}") for i in range(3)]
                KT = [pool.tile([128, N], F16, tag=f"kt{i}", name=f"KT{i}_{b}") for i in range(3)]

                for j in range(NCHUNK):
                    xb = pool.tile([128, KC, CK], F16, tag=f"x{j % 2}", name=f"xb{b}_{j}")
                    nc.sync.dma_start(out=xb[:], in_=xH_d[:, b * NCHUNK + j, :, :])
                    if b == 0 and j == 0:
                        # weights stream in per K-tile behind the first chunk
                        for k in range(KC):
                            nc.sync.dma_start(out=wall[:, k, :], in_=wall_r[:, k, :])
                        nc.sync.dma_start(out=wp[:], in_=wp_d.rearrange("(g p) f -> p g f", p=128))
                    if b > 0 and j == 1:
                        # previous batch's last normalize, deferred across the
                        # phase boundary so its ACT chain never stalls PE
                        flush_pending(0)
                        flush_pending(1)
                    cj = slice(j * CK, (j + 1) * CK)
                    tQ = ps.tile([128, 1024], F32, tag="T0", name=f"tQ{b}_{j}")
                    tK = ps.tile([128, 1024], F32, tag="T1", name=f"tK{b}_{j}")
                    tM = ps.tile([128, 512], F32, tag="bcb", name=f"tM{b}_{j}")
                    tMs = ps.tile([128, 512], F32, tag="ypb", name=f"tMs{b}_{j}")
                    tV = ps.tile([128, 1024], F32, tag="T2", name=f"tV{b}_{j}")
                    for K in range(KC):
                        st, sp = K == 0, K == KC - 1
                        nc.tensor.matmul(tQ[:, 0:256], wall[:, K, 0:128], xb[:, K, :],
                                         start=st, stop=sp)
                        nc.tensor.matmul(tQ[:, 512:768], wall[:, K, 128:256], xb[:, K, :],
                                         start=st, stop=sp)
                    with nc.allow_low_precision(reason="fp16 qkv"):
                        nc.vector.tensor_scalar_add(QT[0][:, cj], tQ[:, 0:256], bias_sb[:, 0:1])
                        nc.vector.tensor_scalar_add(QT[1][:, cj], tQ[:, 512:768], bias_sb[:, 1:2])
                    for K in range(KC):
                        st, sp = K == 0, K == KC - 1
                        nc.tensor.matmul(tK[:, 0:256], wall[:, K, 256:384], xb[:, K, :],
                                         start=st, stop=sp)
                        nc.tensor.matmul(tK[:, 512:768], wall[:, K, 384:512], xb[:, K, :],
                                         start=st, stop=sp)
                    with nc.allow_low_precision(reason="fp16 qkv"):
                        nc.vector.tensor_scalar_add(KT[0][:, cj], tK[:, 0:256], bias_sb[:, 3:4])
                        nc.vector.tensor_scalar_add(KT[1][:, cj], tK[:, 512:768], bias_sb[:, 4:5])
                    for K in range(KC):
                        st, sp = K == 0, K == KC - 1
                        nc.tensor.matmul(tM[:, 0:256], wall[:, K, 512:640],
                                         xb[:, K, :], start=st, stop=sp)
                    nc.vector.tensor_copy(tMs[64:128, 0:256], tM[0:64, 0:256])
                    with nc.allow_low_precision(reason="fp16 qkv"):
                        nc.vector.tensor_scalar_add(QT[2][64:128, cj], tMs[64:128, 0:256],
                                                    bias_sb[64:128, 2:3])
                        nc.vector.tensor_scalar_add(KT[2][64:128, cj], tM[64:128, 0:256],
                                                    bias_sb[64:128, 5:6])
                    for K in range(KC):
                        st, sp = K == 0, K == KC - 1
                        nc.tensor.matmul(tV[:, 0:320], xb[:, K, 0:128], wall[:, K, 640:960],
                                         start=st, stop=sp)
                        nc.tensor.matmul(tV[:, 512:832], xb[:, K, 128:256], wall[:, K, 640:960],
                                         start=st, stop=sp)
                    if b == 0:
                        for tv3 in (V3[2 * j], V3[2 * j + 1]):
                            nc.vector.memset(tv3[:, :, :], 0.0)
                            for h in range(HPC):
                                oc = 64 if h % 2 == 0 else 0
                                nc.vector.memset(tv3[:, h, oc:oc + 1], 1.0)
                    with nc.allow_low_precision(reason="fp16 qkv"):
                        for h in range(HPC):
                            dc = 0 if h % 2 == 0 else 64
                            nc.vector.tensor_copy(
                                V3[2 * j][:, h, dc:dc + 64], tV[:, h * 64:(h + 1) * 64])
                            nc.vector.tensor_copy(
                                V3[2 * j + 1][:, h, dc:dc + 64], tV[:, 512 + h * 64:512 + (h + 1) * 64])

                # ------------- phase A + P: attention with interleaved proj -------------
                nc.leave_named_scope(f"phQ{b}", scope_q[0], False)
                scope_a = nc.enter_named_scope(f"phA{b}", False)

                OTb, boffb = OT, boff

                def unit_pair(p, qb, first_after_qb):
                    """S/exp run one 2-kt step ahead of the O matmuls. Deferred
                    normalize of the previous unit flushes at k2==3 / k2==4 so
                    the ACT ln/exp reciprocal has completed and the two bcb
                    broadcasts don't WAR-stall on one psum bank."""
                    qt, kt_, qbs = QT[p], KT[p], slice(qb * QB, (qb + 1) * QB)
                    oAB = ps.tile([128, 1024], F32, tag="T2", name=f"oAB{b}_{p}_{qb}")
                    sc = pool.tile([128, 1024], F32, tag="sc", name=f"sc{b}_{p}_{qb}", bufs=2)
                    lnr = pool.tile([128, 1024], F32, tag="lnr", name=f"ln{b}_{p}_{qb}", bufs=2)
                    rr = pool.tile([128, 1024], F16, tag="rr", name=f"rr{b}_{p}_{qb}", bufs=2)
                    emit_from = 3 if first_after_qb else 1

                    def s_step(k2):
                        pts = []
                        for kt in (2 * k2, 2 * k2 + 1):
                            s = ps.tile([128, 1024], F32, tag=f"T{kt % 2}",
                                        name=f"s{b}_{p}_{qb}_{kt}")
                            ksl = slice(kt * 128, (kt + 1) * 128)
                            nc.tensor.matmul(s[:, 0:512], kt_[0:64, ksl], qt[0:64, qbs],
                                             start=True, stop=True, tile_position=(0, 0))
                            nc.tensor.matmul(s[:, 512:1024], kt_[64:128, ksl], qt[64:128, qbs],
                                             start=True, stop=True, tile_position=(64, 0))
                            p_t = pool.tile([128, 1024], F16, tag="p",
                                            name=f"p{b}_{p}_{qb}_{kt}", bufs=4)
                            with nc.allow_low_precision(reason="fp16 probs"):
                                nc.scalar.activation(p_t[:], s[:], AF.Exp, scale=SCALE)
                            pts.append(p_t)
                        return pts

                    def o_step(k2, pts, last=False):
                        for i, kt in enumerate((2 * k2, 2 * k2 + 1)):
                            st, sp = kt == 0, kt == KT16 - 1
                            nc.tensor.matmul(oAB[:, 0:512], V3[kt][:, 2 * p, :],
                                             pts[i][:, 0:512], start=st, stop=sp)
                            nc.tensor.matmul(oAB[:, 512:1024], V3[kt][:, 2 * p + 1, :],
                                             pts[i][:, 512:1024], start=st, stop=sp)
                        if last:
                            # normalize prologue before the proj emits so the
                            # DVE/ACT chain starts as early as possible
                            nc.vector.tensor_copy(sc[:, :], oAB[:, :])
                            nc.scalar.activation(lnr[0:65, :], sc[0:65, :], AF.Ln)
                            with nc.allow_low_precision(reason="softmax recip"):
                                nc.scalar.activation(rr[0:65, :], lnr[0:65, :], AF.Exp,
                                                     scale=-1.0)
                        if k2 >= emit_from:
                            emit_proj()
                        if last:
                            emit_proj()

                    prev = s_step(0)
                    for k2 in range(1, KT16 // 2):
                        cur = s_step(k2)
                        if k2 == 3:
                            flush_pending(0)
                        if k2 == 4:
                            flush_pending(1)
                        o_step(k2 - 1, prev)
                        prev = cur
                    o_step(KT16 // 2 - 1, prev, last=True)

                    def _norm_a():
                        bcb = ps.tile([128, 512], F32, tag="bcb", name=f"bc{b}_{p}_{qb}a")
                        nc.tensor.matmul(bcb[:, 0:512], ones128[64:65, :], rr[64:65, 0:512],
                                         start=True, stop=True, tile_position=(64, 0))
                        with nc.allow_low_precision(reason="o norm"):
                            nc.vector.tensor_mul(OT[p][0:64, qbs], sc[0:64, 0:512], bcb[0:64, :])

                    def _norm_b():
                        bcb = ps.tile([128, 512], F32, tag="bcb", name=f"bc{b}_{p}_{qb}b")
                        nc.tensor.matmul(bcb[:, 0:512], ones128[0:1, :], rr[0:1, 512:1024],
                                         start=True, stop=True, tile_position=(0, 0))
                        with nc.allow_low_precision(reason="o norm"):
                            nc.vector.tensor_mul(OT[p][64:128, qbs], sc[64:128, 512:1024],
                                                 bcb[64:128, :])

                    pending[0] = _norm_a
                    pending[1] = _norm_b

                def unit_lone(qb):
                    """5th head: the two k-tiles of each 2-kt step share one
                    [128,1024] S tile and a single fused exp."""
                    qbs = slice(qb * QB, (qb + 1) * QB)
                    oC = ps.tile([128, 1024], F32, tag="T2", name=f"oC{b}_{qb}")
                    sc = pool.tile([128, 1024], F32, tag="sc", name=f"scl{b}_{qb}", bufs=2)
                    lnr = pool.tile([128, 1024], F32, tag="lnr", name=f"lnl{b}_{qb}", bufs=2)
                    rr = pool.tile([128, 1024], F16, tag="rr", name=f"rrl{b}_{qb}", bufs=2)

                    def s_step(k2):
                        s = ps.tile([128, 1024], F32, tag=f"T{k2 % 2}", name=f"sl{b}_{qb}_{k2}")
                        for i, kt in enumerate((2 * k2, 2 * k2 + 1)):
                            ksl = slice(kt * 128, (kt + 1) * 128)
                            nc.tensor.matmul(s[:, 512 * i:512 * i + 512],
                                             KT[2][64:128, ksl], QT[2][64:128, qbs],
                                             start=True, stop=True, tile_position=(64, 0))
                        p_t = pool.tile([128, 1024], F16, tag="p", name=f"pl{b}_{qb}_{k2}", bufs=4)
                        with nc.allow_low_precision(reason="fp16 probs"):
                            nc.scalar.activation(p_t[:], s[:], AF.Exp, scale=SCALE)
                        return p_t

                    def o_step(k2, p_t, last=False):
                        for i, kt in enumerate((2 * k2, 2 * k2 + 1)):
                            st, sp = kt == 0, kt == KT16 - 1
                            nc.tensor.matmul(oC[:, 0:512], V3[kt][:, 4, :],
                                             p_t[:, 512 * i:512 * i + 512], start=st, stop=sp)
                        if last:
                            nc.vector.tensor_copy(sc[:, 0:512], oC[:, 0:512])
                            nc.scalar.activation(lnr[0:65, 0:512], sc[0:65, 0:512], AF.Ln)
                            with nc.allow_low_precision(reason="softmax recip"):
                                nc.scalar.activation(rr[0:65, 0:512], lnr[0:65, 0:512],
                                                     AF.Exp, scale=-1.0)
                        if k2 >= 1:
                            emit_proj()
                        if last:
                            emit_proj()

                    prev = s_step(0)
                    for k2 in range(1, KT16 // 2):
                        cur = s_step(k2)
                        if k2 == 3:
                            flush_pending(0)
                        if k2 == 4:
                            flush_pending(1)
                        o_step(k2 - 1, prev)
                        prev = cur
                    o_step(KT16 // 2 - 1, prev, last=True)

                    def _norm():
                        bcb = ps.tile([128, 512], F32, tag="bcb", name=f"bcl{b}_{qb}")
                        nc.tensor.matmul(bcb[:, 0:512], ones128[64:65, :], rr[64:65, 0:512],
                                         start=True, stop=True, tile_position=(64, 0))
                        with nc.allow_low_precision(reason="o norm"):
                            nc.vector.tensor_mul(OT[2][0:64, qbs], sc[0:64, 0:512], bcb[0:64, :])

                    pending[0] = _norm

                for qb in range(NQB):
                    unit_pair(0, qb, first_after_qb=True)
                    unit_pair(1, qb, first_after_qb=False)
                    unit_lone(qb)
                    for t in range(qb * 4, qb * 4 + 4):
                        for n in range(5):
                            proj_q.append((t, n, OTb, boffb))
                nc.leave_named_scope(f"phA{b}", scope_a[0], False)

            # drain: the first two matmuls of each group don't need OT[2]
            # (written by the still-pending last normalize), so prefix them
            # onto 3 rotating banks while the normalize chain completes.
            # bcb stays reserved for the flush broadcast (reusing it for a
            # prefix would deadlock: new-tile WAR would make the broadcast
            # wait on a finish that needs the broadcast).
            groups = list(proj_q)
            proj_q.clear()
            drain_tiles = {}

            def drain_prefix(g):
                t, n, OTq, boffq = groups[g]
                tsl = slice(t * 128, (t + 1) * 128)
                nsl = slice(n * 512, (n + 1) * 512)
                yp = ps.tile([128, 512], F32, tag=["ypb", "T0", "T1"][g % 3],
                             name=f"dyp{g}")
                nc.tensor.matmul(yp[:, 0:512], OTq[0][:, tsl], wp[:, 0, nsl],
                                 start=True, stop=False)
                nc.tensor.matmul(yp[:, 0:512], OTq[1][:, tsl], wp[:, 1, nsl],
                                 start=False, stop=False)
                drain_tiles[g] = yp

            def drain_finish(g):
                t, n, OTq, boffq = groups[g]
                tsl = slice(t * 128, (t + 1) * 128)
                nsl = slice(n * 512, (n + 1) * 512)
                yp = drain_tiles.pop(g)
                nc.tensor.matmul(yp[:, 0:512], OTq[2][:, tsl], wp[:, 2, nsl],
                                 start=False, stop=True)
                yrow_put(t, n, boffq, yp)

            for g in range(min(3, len(groups))):
                drain_prefix(g)
            flush_pending(0)
            flush_pending(1)
            for g in range(len(groups)):
                drain_finish(g)
                if g + 3 < len(groups):
                    drain_prefix(g + 3)
    return nc


def kernel(x, w_qkv, b_qkv, w_proj, b_proj):
    x = np.asarray(x, np.float32)
    w_qkv = np.asarray(w_qkv, np.float32)
    b_qkv = np.asarray(b_qkv, np.float32)
    w_proj = np.asarray(w_proj, np.float32)
    b_proj = np.asarray(b_proj, np.float32)

    if "nc" not in _CACHE:
        nc = _build()
        nc.compile()
        _CACHE["nc"] = nc
    nc = _CACHE["nc"]

    # [128, chunk, K-tile, token]: each chunk loads as 128 contiguous 10KB
    # descriptor lines instead of 2560 x 512B (DMA is descriptor-rate bound)
    xH = np.ascontiguousarray(
        x.reshape(TOK // CK, CK, KC, 128).transpose(3, 0, 2, 1).astype(np.float16))
    in_maps = []
    for c in range(NCORES):
        f0 = c * HPC * D                                       # 320*c
        qcols = slice(f0, f0 + HPC * D)
        wq = w_qkv[:, qcols]
        wk = w_qkv[:, C + f0: C + f0 + HPC * D]
        wv = w_qkv[:, 2 * C + f0: 2 * C + f0 + HPC * D]
        # [q0..q3 (256) | k0..k3 (256) | q4|k4 (128) | v (320)]
        wall = np.concatenate([wq[:, 0:256], wk[:, 0:256],
                               wq[:, 256:320], wk[:, 256:320], wv], axis=1)
        wp = np.zeros((384, C), np.float16)
        wp[0:320] = w_proj[f0:f0 + HPC * D, :]
        bias = np.zeros((128, 6), np.float32)
        bq = b_qkv[qcols]
        bk = b_qkv[C + f0: C + f0 + HPC * D]
        bias[:, 0] = bq[0:128]
        bias[:, 1] = bq[128:256]
        bias[64:128, 2] = bq[256:320]
        bias[:, 3] = bk[0:128]
        bias[:, 4] = bk[128:256]
        bias[64:128, 5] = bk[256:320]
        in_maps.append({"xH": xH, "wall": np.ascontiguousarray(wall.astype(np.float16)),
                        "wp": wp, "bias": bias})

    _CACHE["in_maps"] = in_maps
    res = run_bass_kernel_spmd(nc, in_maps, core_ids=list(range(NCORES)))
    _CACHE["results"] = res.results
    y = np.zeros((TOK, C), np.float32)
    for c in range(NCORES):
        y += res.results[c]["y"].astype(np.float32)
    bias_eff = b_proj + b_qkv[2 * C:] @ w_proj                 # v-bias folded through proj
    y += bias_eff
    return y.reshape(B, N, C).astype(np.float32)


# revision 8
# speedup vs baseline: 1.0076x; 1.0076x over previous
"""Multi-head attention block (B=2, N=2048, C=2560, H=40, D=64) on 8 TRN2 NeuronCores.

Sharding: tensor-parallel over heads - core c owns heads 5c..5c+4 for both
batch elements. Each core computes qkv for its heads (full token range),
attention, and its partial contribution to the output projection; the host
sums the 8 partials and adds the output bias (v-bias folded through w_proj).

Performance history: 974us (fp32r baseline) -> 734us. The kernel is within
~5% of the two-engine floor: PE busy ~690us of the span, and in phase A the
Activation engine (softmax exp, ~189us/batch) nearly matches PE (~200us/batch),
so further PE savings would hit the exp floor. fp8 DoubleRow was analyzed and
rejected: e4m3's 3-bit mantissa gives ~3-5% relative error on S/O (dot-product
relative error does not average down), vs the 2e-2 gate.

Key design points:
  - x / w_qkv / w_proj all fp16 on device (same 1 cycle/row PE rate as fp32r,
    half the DMA + SBUF). Weights loaded ONCE, per-K-tile, behind the first
    x chunk; x is host-packed [128, chunk, K, tok] so each chunk is 128
    contiguous 10KB DMA lines (the DMA engines are descriptor-rate bound:
    512B lines cost ~3x the bandwidth-ideal time).
  - ~2us of dummy matmuls at t=0 trip the HAM activity window so phase-Q
    starts at 2.4GHz.
  - V tiles padded to 128 weight columns per head: O matmuls get fast-weight
    -load (-80ns each). Even heads: dims at cols 0:64, softmax-denominator
    ones column at col 64 (denominator -> psum partition 64). Odd heads:
    dims at cols 64:128, ones at col 0 (denominator -> partition 0), so the
    odd head's output lands at partitions 64:128 and the normalize needs no
    psum partition-shift. One ACT ln instruction over partitions 0:65 covers
    both denominator rows (cost is free-size only; ln of the garbage rows
    produces unused NaNs).
  - Softmax 1/denominator = ACT exp(-ln d): both funcs live in act table 6,
    loaded manually once (the insert pass would otherwise thrash tables:
    greedy first-match sends Exp to table 0). The old DVE reciprocal was an
    iterative 8-cyc/elem op - 3.3us per row on the critical path.
  - Normalize chain per unit: copy oAB psum->SBUF scratch (frees the bank),
    ln/exp on ACT, PE ones-broadcast (M=128, FWL), then DVE multiplies
    reading scratch(SBUF) x broadcast(PSUM). The two broadcasts flush at
    k2==3 / k2==4 of the NEXT unit (bcb bank WAR + gives the ACT chain time);
    the last unit of a batch flushes inside the next phase (phQ of b+1, or
    interleaved with the drain prefixes).
  - Lone 5th head: two k-tiles share one [128,1024] S tile and one fused exp.
  - Output projection interleaved into attention k-loops (skipping the first
    slots after a q-block boundary while OT settles); y written fp16 as
    [128, 2560] row strips (one 5KB-line DMA per token tile, alternating the
    sync / gpsimd DMA rings) and summed across cores on the host.
  - Final drain pipelines each group's two OT[0]/OT[1] matmuls ahead of the
    last normalize flush on 3 rotating banks (bcb reserved: reusing it for a
    prefix would deadlock against the flush broadcast).

Hardware constraints baked in (discovered empirically):
  - matmul start=True clears has_written for the WHOLE psum bank: one
    accumulation group per bank.
  - DVE/ACT cannot shift partitions psum->sbuf, but DVE CAN shift
    partitions when writing PSUM.
  - K<128 matmuls need explicit tile_position or they run ~7x slow.
  - FWL (fast weight load) needs exactly 128 weight columns.
  - tensor_tensor on DVE may read one PSUM + one SBUF operand.
  - DVE cost is free-size x cycle, partition-count independent; reciprocal
    is 8 cycles/element.
  - Engine clocks vary run-to-run (P0 power state): ~20% of runs execute
    at PE 2.0GHz instead of 2.4GHz, inflating everything uniformly.
"""
import numpy as np

import concourse.bacc as bacc
import concourse.mybir as mybir
import concourse.tile as tile
from concourse.bass_utils import run_bass_kernel_spmd

F32 = mybir.dt.float32
F16 = mybir.dt.float16
AF = mybir.ActivationFunctionType

B, N, C = 2, 2048, 2560
H, D = 40, 64
NCORES = 8
HPC = H // NCORES            # 5 heads per core
SCALE = D ** -0.5
TOK = B * N                  # 4096
CK = 512                     # token chunk in phase Q
NCHUNK = N // CK             # 4 chunks per batch
KT16 = N // 128              # 16 k-tiles per batch
QB = 512                     # q-block
NQB = N // QB                # 4 q-blocks
KC = C // 128                # 20 contraction tiles
ACT_TABLE_LN_EXP = 6         # natural_log_exp_and_others

_CACHE = {}


def _build():
    nc = bacc.Bacc("TRN2", target_bir_lowering=False, debug=False, num_devices=NCORES)
    xH_d = nc.dram_tensor("xH", [128, TOK // CK, KC, CK], F16, kind="ExternalInput")
    wall_d = nc.dram_tensor("wall", [C, 960], F16, kind="ExternalInput")   # q|k (640) + v (320)
    wp_d = nc.dram_tensor("wp", [384, C], F16, kind="ExternalInput")       # padded 320->384
    bias_d = nc.dram_tensor("bias", [128, 6], F32, kind="ExternalInput")   # per-ft qk bias
    y_d = nc.dram_tensor("y", [TOK, C], F16, kind="ExternalOutput")

    with tile.TileContext(nc) as tc:
        with (
            tc.tile_pool(name="sb", bufs=1) as pool,
            tc.tile_pool(name="ps", bufs=1, space="PSUM") as ps,
        ):
            # ln+exp share act table 6; one manual load, the insert pass
            # then sees every Exp/Ln served and adds nothing.
            nc.scalar.add_instruction(mybir.InstLoadActFuncSet(
                name=nc.get_next_instruction_name(),
                act_func_set_id=ACT_TABLE_LN_EXP, ins=[], outs=[]))

            ones128 = pool.tile([128, 128], F16, name="ones128")
            nc.vector.memset(ones128[:], 1.0)
            bias_sb = pool.tile([128, 6], F32, name="bias_sb")
            nc.sync.dma_start(out=bias_sb[:], in_=bias_d[:])

            # ~5us of dummy matmuls while the first DMAs land: trips the HAM
            # activity window so phase-Q matmuls start at 2.4GHz.
            warm = ps.tile([128, 512], F32, tag="ypb", name="warm")
            for i in range(16):
                nc.tensor.matmul(warm[:, 0:128], ones128[:, :], ones128[:, :],
                                 start=(i == 0), stop=(i == 15))

            # V tiles: [128 tok, HPC heads x 128 cols]. Even heads: dims at
            # 0:64, ones col 64. Odd heads: dims at 64:128, ones col 0.
            # Zero + ones init is interleaved into batch-0 chunks (a big
            # upfront DVE memset burst delayed chunk-0 bias adds).
            V = [pool.tile([128, HPC * 128], F16, tag=f"v{i}", name=f"V{i}")
                 for i in range(KT16)]
            V3 = [t.rearrange("p (h e) -> p h e", h=HPC) for t in V]

            wall = pool.tile([128, KC, 960], F16, tag="w", name="wall")
            wall_r = wall_d.rearrange("(t p) f -> p t f", p=128)
            wp = pool.tile([128, 3, C], F16, tag="wpt", name="wp")

            OT = [pool.tile([128, N], F16, tag=f"ot{i}", name=f"OT{i}") for i in range(3)]
            nc.vector.memset(OT[2][64:128, :], 0.0)

            proj_q = []
            yrow_state = {}

            def yrow_put(t, n, boffq, yp):
                """CAST the psum slice into a [128, C] row strip; one 5KB-line
                DMA per t-tile (per-slice 1KB lines were descriptor-bound)."""
                key = (boffq, t)
                if key not in yrow_state:
                    yrow_state[key] = [pool.tile([128, C], F16, tag="y",
                                                 name=f"yrow{boffq}_{t}", bufs=2), 0]
                y_row, cnt = yrow_state[key]
                with nc.allow_low_precision(reason="fp16 y"):
                    nc.vector.tensor_copy(y_row[:, n * 512:(n + 1) * 512], yp[:, 0:512])
                yrow_state[key][1] = cnt + 1
                if cnt + 1 == 5:
                    # alternate rings: gpsimd (Pool) is otherwise idle, and two
                    # rings keep consecutive strip writes concurrent (the tail
                    # strips were serializing on the sync ring)
                    eng = nc.gpsimd if t % 2 else nc.sync
                    eng.dma_start(
                        out=y_d[boffq + t * 128: boffq + (t + 1) * 128, :], in_=y_row[:])
                    del yrow_state[key]

            def emit_proj():
                if not proj_q:
                    return
                t, n, OTq, boffq = proj_q.pop(0)
                tsl = slice(t * 128, (t + 1) * 128)
                nsl = slice(n * 512, (n + 1) * 512)
                yp = ps.tile([128, 512], F32, tag="ypb", name=f"yp{boffq}_{t}_{n}")
                nc.tensor.matmul(yp[:, 0:512], OTq[0][:, tsl], wp[:, 0, nsl],
                                 start=True, stop=False)
                nc.tensor.matmul(yp[:, 0:512], OTq[1][:, tsl], wp[:, 1, nsl],
                                 start=False, stop=False)
                nc.tensor.matmul(yp[:, 0:512], OTq[2][:, tsl], wp[:, 2, nsl],
                                 start=False, stop=True)
                yrow_put(t, n, boffq, yp)

            pending = [None, None]

            def flush_pending(i):
                if pending[i] is not None:
                    pending[i]()
                    pending[i] = None

            for b in range(B):
                boff = b * N

                # ---------------- phase Q: qkv projections ----------------
                scope_q = nc.enter_named_scope(f"phQ{b}", False)
                QT = [pool.tile([128, N], F16, tag=f"qt{i}", name=f"QT{i}_{b}") for i in range(3)]
                KT = [pool.tile([128, N], F16, tag=f"kt{i}", name=f"KT{i}_{b}") for i in range(3)]

                for j in range(NCHUNK):
                    xb = pool.tile([128, KC, CK], F16, tag=f"x{j % 2}", name=f"xb{b}_{j}")
                    nc.sync.dma_start(out=xb[:], in_=xH_d[:, b * NCHUNK + j, :, :])
                    if b == 0 and j == 0:
                        # weights stream in per K-tile behind the first chunk
                        for k in range(KC):
                            nc.sync.dma_start(out=wall[:, k, :], in_=wall_r[:, k, :])
                        nc.sync.dma_start(out=wp[:], in_=wp_d.rearrange("(g p) f -> p g f", p=128))
                    if b > 0 and j == 1:
                        # previous batch's last normalize, deferred across the
                        # phase boundary so its ACT chain never stalls PE
                        flush_pending(0)
                        flush_pending(1)
                    cj = slice(j * CK, (j + 1) * CK)
                    tQ = ps.tile([128, 1024], F32, tag="T0", name=f"tQ{b}_{j}")
                    tK = ps.tile([128, 1024], F32, tag="T1", name=f"tK{b}_{j}")
                    tM = ps.tile([128, 512], F32, tag="bcb", name=f"tM{b}_{j}")
                    tMs = ps.tile([128, 512], F32, tag="ypb", name=f"tMs{b}_{j}")

                    def v_round(r):
                        """V for tokens [r*256, r*256+256) of this chunk; its own
                        20-K accumulation pass so tV stays within 2 psum banks.
                        Interleaved between Q/K/M loops so the T2 WAR on the
                        previous round's DVE copies stays hidden."""
                        kt = 4 * j + 2 * r
                        tV = ps.tile([128, 1024], F32, tag="T2", name=f"tV{b}_{j}_{r}")
                        t0 = r * 256
                        for K in range(KC):
                            st, sp = K == 0, K == KC - 1
                            nc.tensor.matmul(tV[:, 0:320], xb[:, K, t0:t0 + 128],
                                             wall[:, K, 640:960], start=st, stop=sp)
                            nc.tensor.matmul(tV[:, 512:832], xb[:, K, t0 + 128:t0 + 256],
                                             wall[:, K, 640:960], start=st, stop=sp)
                        if b == 0:
                            for tv3 in (V3[kt], V3[kt + 1]):
                                nc.vector.memset(tv3[:, :, :], 0.0)
                                for h in range(HPC):
                                    oc = 64 if h % 2 == 0 else 0
                                    nc.vector.memset(tv3[:, h, oc:oc + 1], 1.0)
                        with nc.allow_low_precision(reason="fp16 qkv"):
                            for h in range(HPC):
                                dc = 0 if h % 2 == 0 else 64
                                nc.vector.tensor_copy(
                                    V3[kt][:, h, dc:dc + 64], tV[:, h * 64:(h + 1) * 64])
                                nc.vector.tensor_copy(
                                    V3[kt + 1][:, h, dc:dc + 64],
                                    tV[:, 512 + h * 64:512 + (h + 1) * 64])

                    for K in range(KC):
                        st, sp = K == 0, K == KC - 1
                        nc.tensor.matmul(tQ[:, 0:512], wall[:, K, 0:128], xb[:, K, :],
                                         start=st, stop=sp)
                        nc.tensor.matmul(tQ[:, 512:1024], wall[:, K, 128:256], xb[:, K, :],
                                         start=st, stop=sp)
                    with nc.allow_low_precision(reason="fp16 qkv"):
                        nc.vector.tensor_scalar_add(QT[0][:, cj], tQ[:, 0:512], bias_sb[:, 0:1])
                        nc.vector.tensor_scalar_add(QT[1][:, cj], tQ[:, 512:1024], bias_sb[:, 1:2])
                    v_round(0)
                    for K in range(KC):
                        st, sp = K == 0, K == KC - 1
                        nc.tensor.matmul(tK[:, 0:512], wall[:, K, 256:384], xb[:, K, :],
                                         start=st, stop=sp)
                        nc.tensor.matmul(tK[:, 512:1024], wall[:, K, 384:512], xb[:, K, :],
                                         start=st, stop=sp)
                    with nc.allow_low_precision(reason="fp16 qkv"):
                        nc.vector.tensor_scalar_add(KT[0][:, cj], tK[:, 0:512], bias_sb[:, 3:4])
                        nc.vector.tensor_scalar_add(KT[1][:, cj], tK[:, 512:1024], bias_sb[:, 4:5])
                    v_round(1)
                    for K in range(KC):
                        st, sp = K == 0, K == KC - 1
                        nc.tensor.matmul(tM[:, 0:512], wall[:, K, 512:640],
                                         xb[:, K, :], start=st, stop=sp)
                    nc.vector.tensor_copy(tMs[64:128, 0:512], tM[0:64, 0:512])
                    with nc.allow_low_precision(reason="fp16 qkv"):
                        nc.vector.tensor_scalar_add(QT[2][64:128, cj], tMs[64:128, 0:512],
                                                    bias_sb[64:128, 2:3])
                        nc.vector.tensor_scalar_add(KT[2][64:128, cj], tM[64:128, 0:512],
                                                    bias_sb[64:128, 5:6])

                # ------------- phase A + P: attention with interleaved proj -------------
                nc.leave_named_scope(f"phQ{b}", scope_q[0], False)
                scope_a = nc.enter_named_scope(f"phA{b}", False)

                OTb, boffb = OT, boff

                def unit_pair(p, qb, first_after_qb):
                    """S/exp run one 2-kt step ahead of the O matmuls. Deferred
                    normalize of the previous unit flushes at k2==3 / k2==4 so
                    the ACT ln/exp reciprocal has completed and the two bcb
                    broadcasts don't WAR-stall on one psum bank."""
                    qt, kt_, qbs = QT[p], KT[p], slice(qb * QB, (qb + 1) * QB)
                    oAB = ps.tile([128, 1024], F32, tag="T2", name=f"oAB{b}_{p}_{qb}")
                    sc = pool.tile([128, 1024], F32, tag="sc", name=f"sc{b}_{p}_{qb}", bufs=2)
                    lnr = pool.tile([128, 1024], F32, tag="lnr", name=f"ln{b}_{p}_{qb}", bufs=2)
                    rr = pool.tile([128, 1024], F16, tag="rr", name=f"rr{b}_{p}_{qb}", bufs=2)
                    emit_from = 3 if first_after_qb else 1

                    def s_step(k2):
                        pts = []
                        for kt in (2 * k2, 2 * k2 + 1):
                            s = ps.tile([128, 1024], F32, tag=f"T{kt % 2}",
                                        name=f"s{b}_{p}_{qb}_{kt}")
                            ksl = slice(kt * 128, (kt + 1) * 128)
                            nc.tensor.matmul(s[:, 0:512], kt_[0:64, ksl], qt[0:64, qbs],
                                             start=True, stop=True, tile_position=(0, 0))
                            nc.tensor.matmul(s[:, 512:1024], kt_[64:128, ksl], qt[64:128, qbs],
                                             start=True, stop=True, tile_position=(64, 0))
                            p_t = pool.tile([128, 1024], F16, tag="p",
                                            name=f"p{b}_{p}_{qb}_{kt}", bufs=4)
                            with nc.allow_low_precision(reason="fp16 probs"):
                                nc.scalar.activation(p_t[:], s[:], AF.Exp, scale=SCALE)
                            pts.append(p_t)
                        return pts

                    def o_step(k2, pts, last=False):
                        for i, kt in enumerate((2 * k2, 2 * k2 + 1)):
                            st, sp = kt == 0, kt == KT16 - 1
                            nc.tensor.matmul(oAB[:, 0:512], V3[kt][:, 2 * p, :],
                                             pts[i][:, 0:512], start=st, stop=sp)
                            nc.tensor.matmul(oAB[:, 512:1024], V3[kt][:, 2 * p + 1, :],
                                             pts[i][:, 512:1024], start=st, stop=sp)
                        if last:
                            # normalize prologue before the proj emits so the
                            # DVE/ACT chain starts as early as possible
                            nc.vector.tensor_copy(sc[:, :], oAB[:, :])
                            nc.scalar.activation(lnr[0:65, :], sc[0:65, :], AF.Ln)
                            with nc.allow_low_precision(reason="softmax recip"):
                                nc.scalar.activation(rr[0:65, :], lnr[0:65, :], AF.Exp,
                                                     scale=-1.0)
                        if k2 >= emit_from:
                            emit_proj()
                        if last:
                            emit_proj()

                    prev = s_step(0)
                    for k2 in range(1, KT16 // 2):
                        cur = s_step(k2)
                        if k2 == 3:
                            flush_pending(0)
                        if k2 == 4:
                            flush_pending(1)
                        o_step(k2 - 1, prev)
                        prev = cur
                    o_step(KT16 // 2 - 1, prev, last=True)

                    def _norm_a():
                        bcb = ps.tile([128, 512], F32, tag="bcb", name=f"bc{b}_{p}_{qb}a")
                        nc.tensor.matmul(bcb[:, 0:512], ones128[64:65, :], rr[64:65, 0:512],
                                         start=True, stop=True, tile_position=(64, 0))
                        with nc.allow_low_precision(reason="o norm"):
                            nc.vector.tensor_mul(OT[p][0:64, qbs], sc[0:64, 0:512], bcb[0:64, :])

                    def _norm_b():
                        bcb = ps.tile([128, 512], F32, tag="bcb", name=f"bc{b}_{p}_{qb}b")
                        nc.tensor.matmul(bcb[:, 0:512], ones128[0:1, :], rr[0:1, 512:1024],
                                         start=True, stop=True, tile_position=(0, 0))
                        with nc.allow_low_precision(reason="o norm"):
                            nc.vector.tensor_mul(OT[p][64:128, qbs], sc[64:128, 512:1024],
                                                 bcb[64:128, :])

                    pending[0] = _norm_a
                    pending[1] = _norm_b

                def unit_lone(qb):
                    """5th head: the two k-tiles of each 2-kt step share one
                    [128,1024] S tile and a single fused exp."""
                    qbs = slice(qb * QB, (qb + 1) * QB)
                    oC = ps.tile([128, 1024], F32, tag="T2", name=f"oC{b}_{qb}")
                    sc = pool.tile([128, 1024], F32, tag="sc", name=f"scl{b}_{qb}", bufs=2)
                    lnr = pool.tile([128, 1024], F32, tag="lnr", name=f"lnl{b}_{qb}", bufs=2)
                    rr = pool.tile([128, 1024], F16, tag="rr", name=f"rrl{b}_{qb}", bufs=2)

                    def s_step(k2):
                        s = ps.tile([128, 1024], F32, tag=f"T{k2 % 2}", name=f"sl{b}_{qb}_{k2}")
                        for i, kt in enumerate((2 * k2, 2 * k2 + 1)):
                            ksl = slice(kt * 128, (kt + 1) * 128)
                            nc.tensor.matmul(s[:, 512 * i:512 * i + 512],
                                             KT[2][64:128, ksl], QT[2][64:128, qbs],
                                             start=True, stop=True, tile_position=(64, 0))
                        p_t = pool.tile([128, 1024], F16, tag="p", name=f"pl{b}_{qb}_{k2}", bufs=4)
                        with nc.allow_low_precision(reason="fp16 probs"):
                            nc.scalar.activation(p_t[:], s[:], AF.Exp, scale=SCALE)
                        return p_t

                    def o_step(k2, p_t, last=False):
                        for i, kt in enumerate((2 * k2, 2 * k2 + 1)):
                            st, sp = kt == 0, kt == KT16 - 1
                            nc.tensor.matmul(oC[:, 0:512], V3[kt][:, 4, :],
                                             p_t[:, 512 * i:512 * i + 512], start=st, stop=sp)
                        if last:
                            nc.vector.tensor_copy(sc[:, 0:512], oC[:, 0:512])
                            nc.scalar.activation(lnr[0:65, 0:512], sc[0:65, 0:512], AF.Ln)
                            with nc.allow_low_precision(reason="softmax recip"):
                                nc.scalar.activation(rr[0:65, 0:512], lnr[0:65, 0:512],
                                                     AF.Exp, scale=-1.0)
                        if k2 >= 1:
                            emit_proj()
                        if last:
                            emit_proj()

                    prev = s_step(0)
                    for k2 in range(1, KT16 // 2):
                        cur = s_step(k2)
                        if k2 == 3:
                            flush_pending(0)
                        if k2 == 4:
                            flush_pending(1)
                        o_step(k2 - 1, prev)
                        prev = cur
                    o_step(KT16 // 2 - 1, prev, last=True)

                    def _norm():
                        bcb = ps.tile([128, 512], F32, tag="bcb", name=f"bcl{b}_{qb}")
                        nc.tensor.matmul(bcb[:, 0:512], ones128[64:65, :], rr[64:65, 0:512],
                                         start=True, stop=True, tile_position=(64, 0))
                        with nc.allow_low_precision(reason="o norm"):
                            nc.vector.tensor_mul(OT[2][0:64, qbs], sc[0:64, 0:512], bcb[0:64, :])

                    pending[0] = _norm

                for qb in range(NQB):
                    unit_pair(0, qb, first_after_qb=True)
                    unit_pair(1, qb, first_after_qb=False)
                    unit_lone(qb)
                    for t in range(qb * 4, qb * 4 + 4):
                        for n in range(5):
                            proj_q.append((t, n, OTb, boffb))
                nc.leave_named_scope(f"phA{b}", scope_a[0], False)

            # drain: the first two matmuls of each group don't need OT[2]
            # (written by the still-pending last normalize), so prefix them
            # onto 3 rotating banks while the normalize chain completes.
            # bcb stays reserved for the flush broadcast (reusing it for a
            # prefix would deadlock: new-tile WAR would make the broadcast
            # wait on a finish that needs the broadcast).
            groups = list(proj_q)
            proj_q.clear()
            drain_tiles = {}

            def drain_prefix(g):
                t, n, OTq, boffq = groups[g]
                tsl = slice(t * 128, (t + 1) * 128)
                nsl = slice(n * 512, (n + 1) * 512)
                yp = ps.tile([128, 512], F32, tag=["ypb", "T0", "T1"][g % 3],
                             name=f"dyp{g}")
                nc.tensor.matmul(yp[:, 0:512], OTq[0][:, tsl], wp[:, 0, nsl],
                                 start=True, stop=False)
                nc.tensor.matmul(yp[:, 0:512], OTq[1][:, tsl], wp[:, 1, nsl],
                                 start=False, stop=False)
                drain_tiles[g] = yp

            def drain_finish(g):
                t, n, OTq, boffq = groups[g]
                tsl = slice(t * 128, (t + 1) * 128)
                nsl = slice(n * 512, (n + 1) * 512)
                yp = drain_tiles.pop(g)
                nc.tensor.matmul(yp[:, 0:512], OTq[2][:, tsl], wp[:, 2, nsl],
                                 start=False, stop=True)
                yrow_put(t, n, boffq, yp)

            for g in range(min(3, len(groups))):
                drain_prefix(g)
            flush_pending(0)
            flush_pending(1)
            for g in range(len(groups)):
                drain_finish(g)
                if g + 3 < len(groups):
                    drain_prefix(g + 3)
    return nc


def kernel(x, w_qkv, b_qkv, w_proj, b_proj):
    x = np.asarray(x, np.float32)
    w_qkv = np.asarray(w_qkv, np.float32)
    b_qkv = np.asarray(b_qkv, np.float32)
    w_proj = np.asarray(w_proj, np.float32)
    b_proj = np.asarray(b_proj, np.float32)

    if "nc" not in _CACHE:
        nc = _build()
        nc.compile()
        _CACHE["nc"] = nc
    nc = _CACHE["nc"]

    # [128, chunk, K-tile, token]: each chunk loads as 128 contiguous 10KB
    # descriptor lines instead of 2560 x 512B (DMA is descriptor-rate bound)
    xH = np.ascontiguousarray(
        x.reshape(TOK // CK, CK, KC, 128).transpose(3, 0, 2, 1).astype(np.float16))
    in_maps = []
    for c in range(NCORES):
        f0 = c * HPC * D                                       # 320*c
        qcols = slice(f0, f0 + HPC * D)
        wq = w_qkv[:, qcols]
        wk = w_qkv[:, C + f0: C + f0 + HPC * D]
        wv = w_qkv[:, 2 * C + f0: 2 * C + f0 + HPC * D]
        # [q0..q3 (256) | k0..k3 (256) | q4|k4 (128) | v (320)]
        wall = np.concatenate([wq[:, 0:256], wk[:, 0:256],
                               wq[:, 256:320], wk[:, 256:320], wv], axis=1)
        wp = np.zeros((384, C), np.float16)
        wp[0:320] = w_proj[f0:f0 + HPC * D, :]
        bias = np.zeros((128, 6), np.float32)
        bq = b_qkv[qcols]
        bk = b_qkv[C + f0: C + f0 + HPC * D]
        bias[:, 0] = bq[0:128]
        bias[:, 1] = bq[128:256]
        bias[64:128, 2] = bq[256:320]
        bias[:, 3] = bk[0:128]
        bias[:, 4] = bk[128:256]
        bias[64:128, 5] = bk[256:320]
        in_maps.append({"xH": xH, "wall": np.ascontiguousarray(wall.astype(np.float16)),
                        "wp": wp, "bias": bias})

    _CACHE["in_maps"] = in_maps
    res = run_bass_kernel_spmd(nc, in_maps, core_ids=list(range(NCORES)))
    _CACHE["results"] = res.results
    y = np.zeros((TOK, C), np.float32)
    for c in range(NCORES):
        y += res.results[c]["y"].astype(np.float32)
    bias_eff = b_proj + b_qkv[2 * C:] @ w_proj                 # v-bias folded through proj
    y += bias_eff
    return y.reshape(B, N, C).astype(np.float32)


# revision 9
# speedup vs baseline: 1.0163x; 1.0087x over previous
"""Multi-head attention block (B=2, N=2048, C=2560, H=40, D=64) on 8 TRN2 NeuronCores.

Sharding: tensor-parallel over heads - core c owns heads 5c..5c+4 for both
batch elements. Each core computes qkv for its heads (full token range),
attention, and its partial contribution to the output projection; the host
sums the 8 partials and adds the output bias (v-bias folded through w_proj).

Performance history: 974us (fp32r baseline) -> 734us. The kernel is within
~5% of the two-engine floor: PE busy ~690us of the span, and in phase A the
Activation engine (softmax exp, ~189us/batch) nearly matches PE (~200us/batch),
so further PE savings would hit the exp floor. fp8 DoubleRow was analyzed and
rejected: e4m3's 3-bit mantissa gives ~3-5% relative error on S/O (dot-product
relative error does not average down), vs the 2e-2 gate.

Key design points:
  - x / w_qkv / w_proj all fp16 on device (same 1 cycle/row PE rate as fp32r,
    half the DMA + SBUF). Weights loaded ONCE, per-K-tile, behind the first
    x chunk; x is host-packed [128, chunk, K, tok] so each chunk is 128
    contiguous 10KB DMA lines (the DMA engines are descriptor-rate bound:
    512B lines cost ~3x the bandwidth-ideal time).
  - ~2us of dummy matmuls at t=0 trip the HAM activity window so phase-Q
    starts at 2.4GHz.
  - V tiles padded to 128 weight columns per head: O matmuls get fast-weight
    -load (-80ns each). Even heads: dims at cols 0:64, softmax-denominator
    ones column at col 64 (denominator -> psum partition 64). Odd heads:
    dims at cols 64:128, ones at col 0 (denominator -> partition 0), so the
    odd head's output lands at partitions 64:128 and the normalize needs no
    psum partition-shift. One ACT ln instruction over partitions 0:65 covers
    both denominator rows (cost is free-size only; ln of the garbage rows
    produces unused NaNs).
  - Softmax 1/denominator = ACT exp(-ln d): both funcs live in act table 6,
    loaded manually once (the insert pass would otherwise thrash tables:
    greedy first-match sends Exp to table 0). The old DVE reciprocal was an
    iterative 8-cyc/elem op - 3.3us per row on the critical path.
  - Normalize chain per unit: copy oAB psum->SBUF scratch (frees the bank),
    ln/exp on ACT, PE ones-broadcast (M=128, FWL), then DVE multiplies
    reading scratch(SBUF) x broadcast(PSUM). The two broadcasts flush at
    k2==3 / k2==4 of the NEXT unit (bcb bank WAR + gives the ACT chain time);
    the last unit of a batch flushes inside the next phase (phQ of b+1, or
    interleaved with the drain prefixes).
  - Lone 5th head: two k-tiles share one [128,1024] S tile and one fused exp.
  - Output projection interleaved into attention k-loops (skipping the first
    slots after a q-block boundary while OT settles); y written fp16 as
    [128, 2560] row strips (one 5KB-line DMA per token tile, alternating the
    sync / gpsimd DMA rings) and summed across cores on the host.
  - Final drain pipelines each group's two OT[0]/OT[1] matmuls ahead of the
    last normalize flush on 3 rotating banks (bcb reserved: reusing it for a
    prefix would deadlock against the flush broadcast).

Hardware constraints baked in (discovered empirically):
  - matmul start=True clears has_written for the WHOLE psum bank: one
    accumulation group per bank.
  - DVE/ACT cannot shift partitions psum->sbuf, but DVE CAN shift
    partitions when writing PSUM.
  - K<128 matmuls need explicit tile_position or they run ~7x slow.
  - FWL (fast weight load) needs exactly 128 weight columns.
  - tensor_tensor on DVE may read one PSUM + one SBUF operand.
  - DVE cost is free-size x cycle, partition-count independent; reciprocal
    is 8 cycles/element.
  - Engine clocks vary run-to-run (P0 power state): ~20% of runs execute
    at PE 2.0GHz instead of 2.4GHz, inflating everything uniformly.
"""
import numpy as np

import concourse.bacc as bacc
import concourse.mybir as mybir
import concourse.tile as tile
from concourse.bass_utils import run_bass_kernel_spmd

F32 = mybir.dt.float32
F16 = mybir.dt.float16
AF = mybir.ActivationFunctionType

B, N, C = 2, 2048, 2560
H, D = 40, 64
NCORES = 8
HPC = H // NCORES            # 5 heads per core
SCALE = D ** -0.5
TOK = B * N                  # 4096
CK = 512                     # token chunk in phase Q
NCHUNK = N // CK             # 4 chunks per batch
KT16 = N // 128              # 16 k-tiles per batch
QB = 512                     # q-block
NQB = N // QB                # 4 q-blocks
KC = C // 128                # 20 contraction tiles
ACT_TABLE_LN_EXP = 6         # natural_log_exp_and_others

_CACHE = {}


def _build():
    nc = bacc.Bacc("TRN2", target_bir_lowering=False, debug=False, num_devices=NCORES)
    xH_d = nc.dram_tensor("xH", [128, TOK // CK, KC, CK], F16, kind="ExternalInput")
    wall_d = nc.dram_tensor("wall", [C, 960], F16, kind="ExternalInput")   # q|k (640) + v (320)
    wp_d = nc.dram_tensor("wp", [384, C], F16, kind="ExternalInput")       # padded 320->384
    bias_d = nc.dram_tensor("bias", [128, 6], F32, kind="ExternalInput")   # per-ft qk bias
    y_d = nc.dram_tensor("y", [TOK, C], F16, kind="ExternalOutput")

    with tile.TileContext(nc) as tc:
        with (
            tc.tile_pool(name="sb", bufs=1) as pool,
            tc.tile_pool(name="ps", bufs=1, space="PSUM") as ps,
        ):
            # ln+exp share act table 6; one manual load, the insert pass
            # then sees every Exp/Ln served and adds nothing.
            nc.scalar.add_instruction(mybir.InstLoadActFuncSet(
                name=nc.get_next_instruction_name(),
                act_func_set_id=ACT_TABLE_LN_EXP, ins=[], outs=[]))

            ones128 = pool.tile([128, 128], F16, name="ones128")
            nc.vector.memset(ones128[:], 1.0)
            bias_sb = pool.tile([128, 6], F32, name="bias_sb")
            nc.sync.dma_start(out=bias_sb[:], in_=bias_d[:])

            # ~5us of dummy matmuls while the first DMAs land: trips the HAM
            # activity window so phase-Q matmuls start at 2.4GHz.
            warm = ps.tile([128, 512], F32, tag="ypb", name="warm")
            for i in range(16):
                nc.tensor.matmul(warm[:, 0:128], ones128[:, :], ones128[:, :],
                                 start=(i == 0), stop=(i == 15))

            # V tiles: [128 tok, HPC heads x 128 cols]. Even heads: dims at
            # 0:64, ones col 64. Odd heads: dims at 64:128, ones col 0.
            # Zero + ones init is interleaved into batch-0 chunks (a big
            # upfront DVE memset burst delayed chunk-0 bias adds).
            V = [pool.tile([128, HPC * 128], F16, tag=f"v{i}", name=f"V{i}")
                 for i in range(KT16)]
            V3 = [t.rearrange("p (h e) -> p h e", h=HPC) for t in V]

            wall = pool.tile([128, KC, 960], F16, tag="w", name="wall")
            wall_r = wall_d.rearrange("(t p) f -> p t f", p=128)
            wp = pool.tile([128, 3, C], F16, tag="wpt", name="wp")

            OT = [pool.tile([128, N], F16, tag=f"ot{i}", name=f"OT{i}") for i in range(3)]
            nc.vector.memset(OT[2][64:128, :], 0.0)

            proj_q = []
            yrow_state = {}

            def yrow_put(t, n, boffq, yp):
                """CAST the psum slice into a [128, C] row strip; one 5KB-line
                DMA per t-tile (per-slice 1KB lines were descriptor-bound)."""
                key = (boffq, t)
                if key not in yrow_state:
                    yrow_state[key] = [pool.tile([128, C], F16, tag="y",
                                                 name=f"yrow{boffq}_{t}", bufs=2), 0]
                y_row, cnt = yrow_state[key]
                with nc.allow_low_precision(reason="fp16 y"):
                    nc.vector.tensor_copy(y_row[:, n * 512:(n + 1) * 512], yp[:, 0:512])
                yrow_state[key][1] = cnt + 1
                if cnt + 1 == 5:
                    # alternate rings: gpsimd (Pool) is otherwise idle, and two
                    # rings keep consecutive strip writes concurrent (the tail
                    # strips were serializing on the sync ring)
                    eng = nc.gpsimd if t % 2 else nc.sync
                    eng.dma_start(
                        out=y_d[boffq + t * 128: boffq + (t + 1) * 128, :], in_=y_row[:])
                    del yrow_state[key]

            def emit_proj():
                if not proj_q:
                    return
                t, n, OTq, boffq = proj_q.pop(0)
                tsl = slice(t * 128, (t + 1) * 128)
                nsl = slice(n * 512, (n + 1) * 512)
                yp = ps.tile([128, 512], F32, tag="ypb", name=f"yp{boffq}_{t}_{n}")
                nc.tensor.matmul(yp[:, 0:512], OTq[0][:, tsl], wp[:, 0, nsl],
                                 start=True, stop=False)
                nc.tensor.matmul(yp[:, 0:512], OTq[1][:, tsl], wp[:, 1, nsl],
                                 start=False, stop=False)
                nc.tensor.matmul(yp[:, 0:512], OTq[2][:, tsl], wp[:, 2, nsl],
                                 start=False, stop=True)
                yrow_put(t, n, boffq, yp)

            pending = [None, None]

            def flush_pending(i):
                if pending[i] is not None:
                    pending[i]()
                    pending[i] = None

            for b in range(B):
                boff = b * N

                # ---------------- phase Q: qkv projections ----------------
                scope_q = nc.enter_named_scope(f"phQ{b}", False)
                QT = [pool.tile([128, N], F16, tag=f"qt{i}", name=f"QT{i}_{b}") for i in range(3)]
                KT = [pool.tile([128, N], F16, tag=f"kt{i}", name=f"KT{i}_{b}") for i in range(3)]

                for j in range(NCHUNK):
                    xb = pool.tile([128, KC, CK], F16, tag=f"x{j % 2}", name=f"xb{b}_{j}")
                    if b == 0 and j == 0:
                        # chunk 0 in quarters, interleaved with the wall
                        # K-tiles, so the first matmul starts ~13us instead of
                        # waiting for the whole 2.6MB chunk
                        for q in range(4):
                            nc.sync.dma_start(out=xb[:, 5 * q:5 * q + 5, :],
                                              in_=xH_d[:, 0, 5 * q:5 * q + 5, :])
                            for k in range(5 * q, 5 * q + 5):
                                nc.sync.dma_start(out=wall[:, k, :], in_=wall_r[:, k, :])
                        nc.sync.dma_start(out=wp[:], in_=wp_d.rearrange("(g p) f -> p g f", p=128))
                    else:
                        nc.sync.dma_start(out=xb[:], in_=xH_d[:, b * NCHUNK + j, :, :])
                    if b > 0 and j == 1:
                        # previous batch's last normalize, deferred across the
                        # phase boundary so its ACT chain never stalls PE
                        flush_pending(0)
                        flush_pending(1)
                    cj = slice(j * CK, (j + 1) * CK)
                    tQ = ps.tile([128, 1024], F32, tag="T0", name=f"tQ{b}_{j}")
                    tK = ps.tile([128, 1024], F32, tag="T1", name=f"tK{b}_{j}")
                    tM = ps.tile([128, 512], F32, tag="bcb", name=f"tM{b}_{j}")
                    tMs = ps.tile([128, 512], F32, tag="ypb", name=f"tMs{b}_{j}")

                    def v_round(r):
                        """V for tokens [r*256, r*256+256) of this chunk; its own
                        20-K accumulation pass so tV stays within 2 psum banks.
                        Interleaved between Q/K/M loops so the T2 WAR on the
                        previous round's DVE copies stays hidden."""
                        kt = 4 * j + 2 * r
                        tV = ps.tile([128, 1024], F32, tag="T2", name=f"tV{b}_{j}_{r}")
                        t0 = r * 256
                        for K in range(KC):
                            st, sp = K == 0, K == KC - 1
                            nc.tensor.matmul(tV[:, 0:320], xb[:, K, t0:t0 + 128],
                                             wall[:, K, 640:960], start=st, stop=sp)
                            nc.tensor.matmul(tV[:, 512:832], xb[:, K, t0 + 128:t0 + 256],
                                             wall[:, K, 640:960], start=st, stop=sp)
                        if b == 0:
                            for tv3 in (V3[kt], V3[kt + 1]):
                                nc.vector.memset(tv3[:, :, :], 0.0)
                                for h in range(HPC):
                                    oc = 64 if h % 2 == 0 else 0
                                    nc.vector.memset(tv3[:, h, oc:oc + 1], 1.0)
                        with nc.allow_low_precision(reason="fp16 qkv"):
                            for h in range(HPC):
                                dc = 0 if h % 2 == 0 else 64
                                nc.vector.tensor_copy(
                                    V3[kt][:, h, dc:dc + 64], tV[:, h * 64:(h + 1) * 64])
                                nc.vector.tensor_copy(
                                    V3[kt + 1][:, h, dc:dc + 64],
                                    tV[:, 512 + h * 64:512 + (h + 1) * 64])

                    for K in range(KC):
                        st, sp = K == 0, K == KC - 1
                        nc.tensor.matmul(tQ[:, 0:512], wall[:, K, 0:128], xb[:, K, :],
                                         start=st, stop=sp)
                        nc.tensor.matmul(tQ[:, 512:1024], wall[:, K, 128:256], xb[:, K, :],
                                         start=st, stop=sp)
                    with nc.allow_low_precision(reason="fp16 qkv"):
                        nc.vector.tensor_scalar_add(QT[0][:, cj], tQ[:, 0:512], bias_sb[:, 0:1])
                        nc.vector.tensor_scalar_add(QT[1][:, cj], tQ[:, 512:1024], bias_sb[:, 1:2])
                    v_round(0)
                    for K in range(KC):
                        st, sp = K == 0, K == KC - 1
                        nc.tensor.matmul(tK[:, 0:512], wall[:, K, 256:384], xb[:, K, :],
                                         start=st, stop=sp)
                        nc.tensor.matmul(tK[:, 512:1024], wall[:, K, 384:512], xb[:, K, :],
                                         start=st, stop=sp)
                    with nc.allow_low_precision(reason="fp16 qkv"):
                        nc.vector.tensor_scalar_add(KT[0][:, cj], tK[:, 0:512], bias_sb[:, 3:4])
                        nc.vector.tensor_scalar_add(KT[1][:, cj], tK[:, 512:1024], bias_sb[:, 4:5])
                    v_round(1)
                    for K in range(KC):
                        st, sp = K == 0, K == KC - 1
                        nc.tensor.matmul(tM[:, 0:512], wall[:, K, 512:640],
                                         xb[:, K, :], start=st, stop=sp)
                    nc.vector.tensor_copy(tMs[64:128, 0:512], tM[0:64, 0:512])
                    with nc.allow_low_precision(reason="fp16 qkv"):
                        nc.vector.tensor_scalar_add(QT[2][64:128, cj], tMs[64:128, 0:512],
                                                    bias_sb[64:128, 2:3])
                        nc.vector.tensor_scalar_add(KT[2][64:128, cj], tM[64:128, 0:512],
                                                    bias_sb[64:128, 5:6])

                # ------------- phase A + P: attention with interleaved proj -------------
                nc.leave_named_scope(f"phQ{b}", scope_q[0], False)
                scope_a = nc.enter_named_scope(f"phA{b}", False)

                OTb, boffb = OT, boff

                def unit_pair(p, qb, first_after_qb):
                    """S/exp run one 2-kt step ahead of the O matmuls. Deferred
                    normalize of the previous unit flushes at k2==3 / k2==4 so
                    the ACT ln/exp reciprocal has completed and the two bcb
                    broadcasts don't WAR-stall on one psum bank."""
                    qt, kt_, qbs = QT[p], KT[p], slice(qb * QB, (qb + 1) * QB)
                    oAB = ps.tile([128, 1024], F32, tag="T2", name=f"oAB{b}_{p}_{qb}")
                    sc = pool.tile([128, 1024], F32, tag="sc", name=f"sc{b}_{p}_{qb}", bufs=2)
                    lnr = pool.tile([128, 1024], F32, tag="lnr", name=f"ln{b}_{p}_{qb}", bufs=2)
                    rr = pool.tile([128, 1024], F16, tag="rr", name=f"rr{b}_{p}_{qb}", bufs=2)
                    emit_from = 3 if first_after_qb else 1

                    def s_step(k2):
                        pts = []
                        for kt in (2 * k2, 2 * k2 + 1):
                            s = ps.tile([128, 1024], F32, tag=f"T{kt % 2}",
                                        name=f"s{b}_{p}_{qb}_{kt}")
                            ksl = slice(kt * 128, (kt + 1) * 128)
                            nc.tensor.matmul(s[:, 0:512], kt_[0:64, ksl], qt[0:64, qbs],
                                             start=True, stop=True, tile_position=(0, 0))
                            nc.tensor.matmul(s[:, 512:1024], kt_[64:128, ksl], qt[64:128, qbs],
                                             start=True, stop=True, tile_position=(64, 0))
                            p_t = pool.tile([128, 1024], F16, tag="p",
                                            name=f"p{b}_{p}_{qb}_{kt}", bufs=4)
                            with nc.allow_low_precision(reason="fp16 probs"):
                                nc.scalar.activation(p_t[:], s[:], AF.Exp, scale=SCALE)
                            pts.append(p_t)
                        return pts

                    def o_step(k2, pts, last=False):
                        for i, kt in enumerate((2 * k2, 2 * k2 + 1)):
                            st, sp = kt == 0, kt == KT16 - 1
                            nc.tensor.matmul(oAB[:, 0:512], V3[kt][:, 2 * p, :],
                                             pts[i][:, 0:512], start=st, stop=sp)
                            nc.tensor.matmul(oAB[:, 512:1024], V3[kt][:, 2 * p + 1, :],
                                             pts[i][:, 512:1024], start=st, stop=sp)
                        if last:
                            # normalize prologue before the proj emits so the
                            # DVE/ACT chain starts as early as possible
                            nc.vector.tensor_copy(sc[:, :], oAB[:, :])
                            nc.scalar.activation(lnr[0:65, :], sc[0:65, :], AF.Ln)
                            with nc.allow_low_precision(reason="softmax recip"):
                                nc.scalar.activation(rr[0:65, :], lnr[0:65, :], AF.Exp,
                                                     scale=-1.0)
                        if k2 >= emit_from:
                            emit_proj()
                        if last:
                            emit_proj()

                    prev = s_step(0)
                    for k2 in range(1, KT16 // 2):
                        cur = s_step(k2)
                        if k2 == 3:
                            flush_pending(0)
                        if k2 == 4:
                            flush_pending(1)
                        o_step(k2 - 1, prev)
                        prev = cur
                    o_step(KT16 // 2 - 1, prev, last=True)

                    def _norm_a():
                        bcb = ps.tile([128, 512], F32, tag="bcb", name=f"bc{b}_{p}_{qb}a")
                        nc.tensor.matmul(bcb[:, 0:512], ones128[64:65, :], rr[64:65, 0:512],
                                         start=True, stop=True, tile_position=(64, 0))
                        with nc.allow_low_precision(reason="o norm"):
                            nc.vector.tensor_mul(OT[p][0:64, qbs], sc[0:64, 0:512], bcb[0:64, :])

                    def _norm_b():
                        bcb = ps.tile([128, 512], F32, tag="bcb", name=f"bc{b}_{p}_{qb}b")
                        nc.tensor.matmul(bcb[:, 0:512], ones128[0:1, :], rr[0:1, 512:1024],
                                         start=True, stop=True, tile_position=(0, 0))
                        with nc.allow_low_precision(reason="o norm"):
                            nc.vector.tensor_mul(OT[p][64:128, qbs], sc[64:128, 512:1024],
                                                 bcb[64:128, :])

                    pending[0] = _norm_a
                    pending[1] = _norm_b

                def unit_lone(qb):
                    """5th head: the two k-tiles of each 2-kt step share one
                    [128,1024] S tile and a single fused exp."""
                    qbs = slice(qb * QB, (qb + 1) * QB)
                    oC = ps.tile([128, 1024], F32, tag="T2", name=f"oC{b}_{qb}")
                    sc = pool.tile([128, 1024], F32, tag="sc", name=f"scl{b}_{qb}", bufs=2)
                    lnr = pool.tile([128, 1024], F32, tag="lnr", name=f"lnl{b}_{qb}", bufs=2)
                    rr = pool.tile([128, 1024], F16, tag="rr", name=f"rrl{b}_{qb}", bufs=2)

                    def s_step(k2):
                        s = ps.tile([128, 1024], F32, tag=f"T{k2 % 2}", name=f"sl{b}_{qb}_{k2}")
                        for i, kt in enumerate((2 * k2, 2 * k2 + 1)):
                            ksl = slice(kt * 128, (kt + 1) * 128)
                            nc.tensor.matmul(s[:, 512 * i:512 * i + 512],
                                             KT[2][64:128, ksl], QT[2][64:128, qbs],
                                             start=True, stop=True, tile_position=(64, 0))
                        p_t = pool.tile([128, 1024], F16, tag="p", name=f"pl{b}_{qb}_{k2}", bufs=4)
                        with nc.allow_low_precision(reason="fp16 probs"):
                            nc.scalar.activation(p_t[:], s[:], AF.Exp, scale=SCALE)
                        return p_t

                    def o_step(k2, p_t, last=False):
                        for i, kt in enumerate((2 * k2, 2 * k2 + 1)):
                            st, sp = kt == 0, kt == KT16 - 1
                            nc.tensor.matmul(oC[:, 0:512], V3[kt][:, 4, :],
                                             p_t[:, 512 * i:512 * i + 512], start=st, stop=sp)
                        if last:
                            nc.vector.tensor_copy(sc[:, 0:512], oC[:, 0:512])
                            nc.scalar.activation(lnr[0:65, 0:512], sc[0:65, 0:512], AF.Ln)
                            with nc.allow_low_precision(reason="softmax recip"):
                                nc.scalar.activation(rr[0:65, 0:512], lnr[0:65, 0:512],
                                                     AF.Exp, scale=-1.0)
                        if k2 >= 1:
                            emit_proj()
                        if last:
                            emit_proj()

                    prev = s_step(0)
                    for k2 in range(1, KT16 // 2):
                        cur = s_step(k2)
                        if k2 == 3:
                            flush_pending(0)
                        if k2 == 4:
                            flush_pending(1)
                        o_step(k2 - 1, prev)
                        prev = cur
                    o_step(KT16 // 2 - 1, prev, last=True)

                    def _norm():
                        bcb = ps.tile([128, 512], F32, tag="bcb", name=f"bcl{b}_{qb}")
                        nc.tensor.matmul(bcb[:, 0:512], ones128[64:65, :], rr[64:65, 0:512],
                                         start=True, stop=True, tile_position=(64, 0))
                        with nc.allow_low_precision(reason="o norm"):
                            nc.vector.tensor_mul(OT[2][0:64, qbs], sc[0:64, 0:512], bcb[0:64, :])

                    pending[0] = _norm

                for qb in range(NQB):
                    unit_pair(0, qb, first_after_qb=True)
                    unit_pair(1, qb, first_after_qb=False)
                    unit_lone(qb)
                    for t in range(qb * 4, qb * 4 + 4):
                        for n in range(5):
                            proj_q.append((t, n, OTb, boffb))
                nc.leave_named_scope(f"phA{b}", scope_a[0], False)

            # drain: the first two matmuls of each group don't need OT[2]
            # (written by the still-pending last normalize), so prefix them
            # onto 3 rotating banks while the normalize chain completes.
            # bcb stays reserved for the flush broadcast (reusing it for a
            # prefix would deadlock: new-tile WAR would make the broadcast
            # wait on a finish that needs the broadcast).
            groups = list(proj_q)
            proj_q.clear()
            drain_tiles = {}

            def drain_prefix(g):
                t, n, OTq, boffq = groups[g]
                tsl = slice(t * 128, (t + 1) * 128)
                nsl = slice(n * 512, (n + 1) * 512)
                yp = ps.tile([128, 512], F32, tag=["ypb", "T0", "T1"][g % 3],
                             name=f"dyp{g}")
                nc.tensor.matmul(yp[:, 0:512], OTq[0][:, tsl], wp[:, 0, nsl],
                                 start=True, stop=False)
                nc.tensor.matmul(yp[:, 0:512], OTq[1][:, tsl], wp[:, 1, nsl],
                                 start=False, stop=False)
                drain_tiles[g] = yp

            def drain_finish(g):
                t, n, OTq, boffq = groups[g]
                tsl = slice(t * 128, (t + 1) * 128)
                nsl = slice(n * 512, (n + 1) * 512)
                yp = drain_tiles.pop(g)
                nc.tensor.matmul(yp[:, 0:512], OTq[2][:, tsl], wp[:, 2, nsl],
                                 start=False, stop=True)
                yrow_put(t, n, boffq, yp)

            for g in range(min(3, len(groups))):
                drain_prefix(g)
            flush_pending(0)
            flush_pending(1)
            for g in range(len(groups)):
                drain_finish(g)
                if g + 3 < len(groups):
                    drain_prefix(g + 3)
    return nc


def kernel(x, w_qkv, b_qkv, w_proj, b_proj):
    x = np.asarray(x, np.float32)
    w_qkv = np.asarray(w_qkv, np.float32)
    b_qkv = np.asarray(b_qkv, np.float32)
    w_proj = np.asarray(w_proj, np.float32)
    b_proj = np.asarray(b_proj, np.float32)

    if "nc" not in _CACHE:
        nc = _build()
        nc.compile()
        _CACHE["nc"] = nc
    nc = _CACHE["nc"]

    # [128, chunk, K-tile, token]: each chunk loads as 128 contiguous 10KB
    # descriptor lines instead of 2560 x 512B (DMA is descriptor-rate bound)
    xH = np.ascontiguousarray(
        x.reshape(TOK // CK, CK, KC, 128).transpose(3, 0, 2, 1).astype(np.float16))
    in_maps = []
    for c in range(NCORES):
        f0 = c * HPC * D                                       # 320*c
        qcols = slice(f0, f0 + HPC * D)
        wq = w_qkv[:, qcols]
        wk = w_qkv[:, C + f0: C + f0 + HPC * D]
        wv = w_qkv[:, 2 * C + f0: 2 * C + f0 + HPC * D]
        # [q0..q3 (256) | k0..k3 (256) | q4|k4 (128) | v (320)]
        wall = np.concatenate([wq[:, 0:256], wk[:, 0:256],
                               wq[:, 256:320], wk[:, 256:320], wv], axis=1)
        wp = np.zeros((384, C), np.float16)
        wp[0:320] = w_proj[f0:f0 + HPC * D, :]
        bias = np.zeros((128, 6), np.float32)
        bq = b_qkv[qcols]
        bk = b_qkv[C + f0: C + f0 + HPC * D]
        bias[:, 0] = bq[0:128]
        bias[:, 1] = bq[128:256]
        bias[64:128, 2] = bq[256:320]
        bias[:, 3] = bk[0:128]
        bias[:, 4] = bk[128:256]
        bias[64:128, 5] = bk[256:320]
        in_maps.append({"xH": xH, "wall": np.ascontiguousarray(wall.astype(np.float16)),
                        "wp": wp, "bias": bias})

    _CACHE["in_maps"] = in_maps
    res = run_bass_kernel_spmd(nc, in_maps, core_ids=list(range(NCORES)))
    _CACHE["results"] = res.results
    y = np.zeros((TOK, C), np.float32)
    for c in range(NCORES):
        y += res.results[c]["y"].astype(np.float32)
    bias_eff = b_proj + b_qkv[2 * C:] @ w_proj                 # v-bias folded through proj
    y += bias_eff
    return y.reshape(B, N, C).astype(np.float32)


# revision 10
# speedup vs baseline: 1.0210x; 1.0046x over previous
"""Multi-head attention block (B=2, N=2048, C=2560, H=40, D=64) on 8 TRN2 NeuronCores.

Sharding: tensor-parallel over heads - core c owns heads 5c..5c+4 for both
batch elements. Each core computes qkv for its heads (full token range),
attention, and its partial contribution to the output projection; the host
sums the 8 partials and adds the output bias (v-bias folded through w_proj).

Performance history: 974us (fp32r baseline) -> 728us. The kernel is within
~5% of the two-engine floor: PE busy ~690us of the span, and in phase A the
Activation engine (softmax exp, ~189us/batch) nearly matches PE (~200us/batch),
so further PE savings would hit the exp floor. fp8 DoubleRow was analyzed and
rejected: e4m3's 3-bit mantissa gives ~3-5% relative error on S/O (dot-product
relative error does not average down), vs the 2e-2 gate.

Key design points:
  - x / w_qkv / w_proj all fp16 on device (same 1 cycle/row PE rate as fp32r,
    half the DMA + SBUF). Weights loaded ONCE, per-K-tile, interleaved with
    quarter-loads of the first x chunk; x is host-packed [128, chunk, K, tok]
    so each chunk is 128 contiguous 20KB DMA lines (the DMA engines are
    descriptor-rate bound: 512B lines cost ~3x the bandwidth-ideal time).
    Phase Q uses 512-token chunks: Q/K/M matmuls stream 512 moving columns
    (halves instruction count vs 256); V runs as two 256-token accumulation
    rounds interleaved between the Q/K loops (psum bank limit + hides the
    T2-bank WAR on the previous round's DVE copies). Phase Q is 99.8% PE-dense.
  - ~2us of dummy matmuls at t=0 trip the HAM activity window so phase-Q
    starts at 2.4GHz.
  - V tiles padded to 128 weight columns per head: O matmuls get fast-weight
    -load (-80ns each). Even heads: dims at cols 0:64, softmax-denominator
    ones column at col 64 (denominator -> psum partition 64). Odd heads:
    dims at cols 64:128, ones at col 0 (denominator -> partition 0), so the
    odd head's output lands at partitions 64:128 and the normalize needs no
    psum partition-shift. One ACT ln instruction over partitions 0:65 covers
    both denominator rows (cost is free-size only; ln of the garbage rows
    produces unused NaNs).
  - Softmax 1/denominator = ACT exp(-ln d): both funcs live in act table 6,
    loaded manually once (the insert pass would otherwise thrash tables:
    greedy first-match sends Exp to table 0). The old DVE reciprocal was an
    iterative 8-cyc/elem op - 3.3us per row on the critical path.
  - Normalize chain per unit: copy oAB psum->SBUF scratch (frees the bank),
    ln/exp on ACT, PE ones-broadcast (M=128, FWL), then DVE multiplies
    reading scratch(SBUF) x broadcast(PSUM). The two broadcasts flush at
    k2==3 / k2==4 of the NEXT unit (bcb bank WAR + gives the ACT chain time);
    the last unit of a batch flushes inside the next phase (phQ of b+1, or
    interleaved with the drain prefixes).
  - Lone 5th head: two k-tiles share one [128,1024] S tile and one fused exp.
  - Output projection interleaved into attention k-loops (skipping the first
    slots after a q-block boundary while OT settles); y written fp16 as
    [128, 2560] row strips (one 5KB-line DMA per token tile, alternating the
    sync / gpsimd DMA rings) and summed across cores on the host.
  - Final drain pipelines each group's two OT[0]/OT[1] matmuls ahead of the
    last normalize flush on 3 rotating banks (bcb reserved: reusing it for a
    prefix would deadlock against the flush broadcast).

Hardware constraints baked in (discovered empirically):
  - matmul start=True clears has_written for the WHOLE psum bank: one
    accumulation group per bank.
  - DVE/ACT cannot shift partitions psum->sbuf, but DVE CAN shift
    partitions when writing PSUM.
  - K<128 matmuls need explicit tile_position or they run ~7x slow.
  - FWL (fast weight load) needs exactly 128 weight columns.
  - tensor_tensor on DVE may read one PSUM + one SBUF operand.
  - DVE cost is free-size x cycle, partition-count independent; reciprocal
    is 8 cycles/element.
  - Engine clocks vary run-to-run (P0 power state): ~20% of runs execute
    at PE 2.0GHz instead of 2.4GHz, inflating everything uniformly.
"""
import numpy as np

import concourse.bacc as bacc
import concourse.mybir as mybir
import concourse.tile as tile
from concourse.bass_utils import run_bass_kernel_spmd

F32 = mybir.dt.float32
F16 = mybir.dt.float16
AF = mybir.ActivationFunctionType

B, N, C = 2, 2048, 2560
H, D = 40, 64
NCORES = 8
HPC = H // NCORES            # 5 heads per core
SCALE = D ** -0.5
TOK = B * N                  # 4096
CK = 512                     # token chunk in phase Q
NCHUNK = N // CK             # 4 chunks per batch
KT16 = N // 128              # 16 k-tiles per batch
QB = 512                     # q-block
NQB = N // QB                # 4 q-blocks
KC = C // 128                # 20 contraction tiles
ACT_TABLE_LN_EXP = 6         # natural_log_exp_and_others

_CACHE = {}


def _build():
    nc = bacc.Bacc("TRN2", target_bir_lowering=False, debug=False, num_devices=NCORES)
    xH_d = nc.dram_tensor("xH", [128, TOK // CK, KC, CK], F16, kind="ExternalInput")
    wall_d = nc.dram_tensor("wall", [C, 960], F16, kind="ExternalInput")   # q|k (640) + v (320)
    wp_d = nc.dram_tensor("wp", [384, C], F16, kind="ExternalInput")       # padded 320->384
    bias_d = nc.dram_tensor("bias", [128, 6], F32, kind="ExternalInput")   # per-ft qk bias
    y_d = nc.dram_tensor("y", [TOK, C], F16, kind="ExternalOutput")

    with tile.TileContext(nc) as tc:
        with (
            tc.tile_pool(name="sb", bufs=1) as pool,
            tc.tile_pool(name="ps", bufs=1, space="PSUM") as ps,
        ):
            # ln+exp share act table 6; one manual load, the insert pass
            # then sees every Exp/Ln served and adds nothing.
            nc.scalar.add_instruction(mybir.InstLoadActFuncSet(
                name=nc.get_next_instruction_name(),
                act_func_set_id=ACT_TABLE_LN_EXP, ins=[], outs=[]))

            ones128 = pool.tile([128, 128], F16, name="ones128")
            nc.vector.memset(ones128[:], 1.0)
            bias_sb = pool.tile([128, 6], F32, name="bias_sb")
            nc.sync.dma_start(out=bias_sb[:], in_=bias_d[:])

            # ~5us of dummy matmuls while the first DMAs land: trips the HAM
            # activity window so phase-Q matmuls start at 2.4GHz.
            warm = ps.tile([128, 512], F32, tag="ypb", name="warm")
            for i in range(16):
                nc.tensor.matmul(warm[:, 0:128], ones128[:, :], ones128[:, :],
                                 start=(i == 0), stop=(i == 15))

            # V tiles: [128 tok, HPC heads x 128 cols]. Even heads: dims at
            # 0:64, ones col 64. Odd heads: dims at 64:128, ones col 0.
            # Zero + ones init is interleaved into batch-0 chunks (a big
            # upfront DVE memset burst delayed chunk-0 bias adds).
            V = [pool.tile([128, HPC * 128], F16, tag=f"v{i}", name=f"V{i}")
                 for i in range(KT16)]
            V3 = [t.rearrange("p (h e) -> p h e", h=HPC) for t in V]

            wall = pool.tile([128, KC, 960], F16, tag="w", name="wall")
            wall_r = wall_d.rearrange("(t p) f -> p t f", p=128)
            wp = pool.tile([128, 3, C], F16, tag="wpt", name="wp")

            OT = [pool.tile([128, N], F16, tag=f"ot{i}", name=f"OT{i}") for i in range(3)]
            nc.vector.memset(OT[2][64:128, :], 0.0)

            proj_q = []
            yrow_state = {}

            def yrow_put(t, n, boffq, yp):
                """CAST the psum slice into a [128, C] row strip; one 5KB-line
                DMA per t-tile (per-slice 1KB lines were descriptor-bound)."""
                key = (boffq, t)
                if key not in yrow_state:
                    yrow_state[key] = [pool.tile([128, C], F16, tag="y",
                                                 name=f"yrow{boffq}_{t}", bufs=2), 0]
                y_row, cnt = yrow_state[key]
                with nc.allow_low_precision(reason="fp16 y"):
                    nc.vector.tensor_copy(y_row[:, n * 512:(n + 1) * 512], yp[:, 0:512])
                yrow_state[key][1] = cnt + 1
                if cnt + 1 == 5:
                    # alternate rings: gpsimd (Pool) is otherwise idle, and two
                    # rings keep consecutive strip writes concurrent (the tail
                    # strips were serializing on the sync ring)
                    eng = nc.gpsimd if t % 2 else nc.sync
                    eng.dma_start(
                        out=y_d[boffq + t * 128: boffq + (t + 1) * 128, :], in_=y_row[:])
                    del yrow_state[key]

            def emit_proj():
                if not proj_q:
                    return
                t, n, OTq, boffq = proj_q.pop(0)
                tsl = slice(t * 128, (t + 1) * 128)
                nsl = slice(n * 512, (n + 1) * 512)
                yp = ps.tile([128, 512], F32, tag="ypb", name=f"yp{boffq}_{t}_{n}")
                nc.tensor.matmul(yp[:, 0:512], OTq[0][:, tsl], wp[:, 0, nsl],
                                 start=True, stop=False)
                nc.tensor.matmul(yp[:, 0:512], OTq[1][:, tsl], wp[:, 1, nsl],
                                 start=False, stop=False)
                nc.tensor.matmul(yp[:, 0:512], OTq[2][:, tsl], wp[:, 2, nsl],
                                 start=False, stop=True)
                yrow_put(t, n, boffq, yp)

            pending = [None, None]

            def flush_pending(i):
                if pending[i] is not None:
                    pending[i]()
                    pending[i] = None

            for b in range(B):
                boff = b * N

                # ---------------- phase Q: qkv projections ----------------
                scope_q = nc.enter_named_scope(f"phQ{b}", False)
                QT = [pool.tile([128, N], F16, tag=f"qt{i}", name=f"QT{i}_{b}") for i in range(3)]
                KT = [pool.tile([128, N], F16, tag=f"kt{i}", name=f"KT{i}_{b}") for i in range(3)]

                for j in range(NCHUNK):
                    xb = pool.tile([128, KC, CK], F16, tag=f"x{j % 2}", name=f"xb{b}_{j}")
                    if b == 0 and j == 0:
                        # chunk 0 in quarters, interleaved with the wall
                        # K-tiles, so the first matmul starts ~13us instead of
                        # waiting for the whole 2.6MB chunk
                        for q in range(4):
                            nc.sync.dma_start(out=xb[:, 5 * q:5 * q + 5, :],
                                              in_=xH_d[:, 0, 5 * q:5 * q + 5, :])
                            for k in range(5 * q, 5 * q + 5):
                                nc.sync.dma_start(out=wall[:, k, :], in_=wall_r[:, k, :])
                        nc.sync.dma_start(out=wp[:], in_=wp_d.rearrange("(g p) f -> p g f", p=128))
                    else:
                        nc.sync.dma_start(out=xb[:], in_=xH_d[:, b * NCHUNK + j, :, :])
                    if b > 0 and j == 1:
                        # previous batch's last normalize, deferred across the
                        # phase boundary so its ACT chain never stalls PE
                        flush_pending(0)
                        flush_pending(1)
                    cj = slice(j * CK, (j + 1) * CK)
                    tQ = ps.tile([128, 1024], F32, tag="T0", name=f"tQ{b}_{j}")
                    tK = ps.tile([128, 1024], F32, tag="T1", name=f"tK{b}_{j}")
                    tM = ps.tile([128, 512], F32, tag="bcb", name=f"tM{b}_{j}")
                    tMs = ps.tile([128, 512], F32, tag="ypb", name=f"tMs{b}_{j}")

                    def v_round(r):
                        """V for tokens [r*256, r*256+256) of this chunk; its own
                        20-K accumulation pass so tV stays within 2 psum banks.
                        Interleaved between Q/K/M loops so the T2 WAR on the
                        previous round's DVE copies stays hidden."""
                        kt = 4 * j + 2 * r
                        tV = ps.tile([128, 1024], F32, tag="T2", name=f"tV{b}_{j}_{r}")
                        t0 = r * 256
                        for K in range(KC):
                            st, sp = K == 0, K == KC - 1
                            nc.tensor.matmul(tV[:, 0:320], xb[:, K, t0:t0 + 128],
                                             wall[:, K, 640:960], start=st, stop=sp)
                            nc.tensor.matmul(tV[:, 512:832], xb[:, K, t0 + 128:t0 + 256],
                                             wall[:, K, 640:960], start=st, stop=sp)
                        if b == 0:
                            for tv3 in (V3[kt], V3[kt + 1]):
                                nc.vector.memset(tv3[:, :, :], 0.0)
                                for h in range(HPC):
                                    oc = 64 if h % 2 == 0 else 0
                                    nc.vector.memset(tv3[:, h, oc:oc + 1], 1.0)
                        with nc.allow_low_precision(reason="fp16 qkv"):
                            for h in range(HPC):
                                dc = 0 if h % 2 == 0 else 64
                                nc.vector.tensor_copy(
                                    V3[kt][:, h, dc:dc + 64], tV[:, h * 64:(h + 1) * 64])
                                nc.vector.tensor_copy(
                                    V3[kt + 1][:, h, dc:dc + 64],
                                    tV[:, 512 + h * 64:512 + (h + 1) * 64])

                    for K in range(KC):
                        st, sp = K == 0, K == KC - 1
                        nc.tensor.matmul(tQ[:, 0:512], wall[:, K, 0:128], xb[:, K, :],
                                         start=st, stop=sp)
                        nc.tensor.matmul(tQ[:, 512:1024], wall[:, K, 128:256], xb[:, K, :],
                                         start=st, stop=sp)
                    with nc.allow_low_precision(reason="fp16 qkv"):
                        nc.vector.tensor_scalar_add(QT[0][:, cj], tQ[:, 0:512], bias_sb[:, 0:1])
                        nc.vector.tensor_scalar_add(QT[1][:, cj], tQ[:, 512:1024], bias_sb[:, 1:2])
                    v_round(0)
                    for K in range(KC):
                        st, sp = K == 0, K == KC - 1
                        nc.tensor.matmul(tK[:, 0:512], wall[:, K, 256:384], xb[:, K, :],
                                         start=st, stop=sp)
                        nc.tensor.matmul(tK[:, 512:1024], wall[:, K, 384:512], xb[:, K, :],
                                         start=st, stop=sp)
                    with nc.allow_low_precision(reason="fp16 qkv"):
                        nc.vector.tensor_scalar_add(KT[0][:, cj], tK[:, 0:512], bias_sb[:, 3:4])
                        nc.vector.tensor_scalar_add(KT[1][:, cj], tK[:, 512:1024], bias_sb[:, 4:5])
                    v_round(1)
                    for K in range(KC):
                        st, sp = K == 0, K == KC - 1
                        nc.tensor.matmul(tM[:, 0:512], wall[:, K, 512:640],
                                         xb[:, K, :], start=st, stop=sp)
                    nc.vector.tensor_copy(tMs[64:128, 0:512], tM[0:64, 0:512])
                    with nc.allow_low_precision(reason="fp16 qkv"):
                        nc.vector.tensor_scalar_add(QT[2][64:128, cj], tMs[64:128, 0:512],
                                                    bias_sb[64:128, 2:3])
                        nc.vector.tensor_scalar_add(KT[2][64:128, cj], tM[64:128, 0:512],
                                                    bias_sb[64:128, 5:6])

                # ------------- phase A + P: attention with interleaved proj -------------
                nc.leave_named_scope(f"phQ{b}", scope_q[0], False)
                scope_a = nc.enter_named_scope(f"phA{b}", False)

                OTb, boffb = OT, boff

                def unit_pair(p, qb, first_after_qb):
                    """S/exp run one 2-kt step ahead of the O matmuls. Deferred
                    normalize of the previous unit flushes at k2==3 / k2==4 so
                    the ACT ln/exp reciprocal has completed and the two bcb
                    broadcasts don't WAR-stall on one psum bank."""
                    qt, kt_, qbs = QT[p], KT[p], slice(qb * QB, (qb + 1) * QB)
                    oAB = ps.tile([128, 1024], F32, tag="T2", name=f"oAB{b}_{p}_{qb}")
                    sc = pool.tile([128, 1024], F32, tag="sc", name=f"sc{b}_{p}_{qb}", bufs=2)
                    lnr = pool.tile([128, 1024], F32, tag="lnr", name=f"ln{b}_{p}_{qb}", bufs=2)
                    rr = pool.tile([128, 1024], F16, tag="rr", name=f"rr{b}_{p}_{qb}", bufs=2)
                    emit_from = 3 if first_after_qb else 1

                    def s_step(k2):
                        pts = []
                        for kt in (2 * k2, 2 * k2 + 1):
                            s = ps.tile([128, 1024], F32, tag=f"T{kt % 2}",
                                        name=f"s{b}_{p}_{qb}_{kt}")
                            ksl = slice(kt * 128, (kt + 1) * 128)
                            nc.tensor.matmul(s[:, 0:512], kt_[0:64, ksl], qt[0:64, qbs],
                                             start=True, stop=True, tile_position=(0, 0))
                            nc.tensor.matmul(s[:, 512:1024], kt_[64:128, ksl], qt[64:128, qbs],
                                             start=True, stop=True, tile_position=(64, 0))
                            p_t = pool.tile([128, 1024], F16, tag="p",
                                            name=f"p{b}_{p}_{qb}_{kt}", bufs=4)
                            with nc.allow_low_precision(reason="fp16 probs"):
                                nc.scalar.activation(p_t[:], s[:], AF.Exp, scale=SCALE)
                            pts.append(p_t)
                        return pts

                    def o_step(k2, pts, last=False):
                        for i, kt in enumerate((2 * k2, 2 * k2 + 1)):
                            st, sp = kt == 0, kt == KT16 - 1
                            nc.tensor.matmul(oAB[:, 0:512], V3[kt][:, 2 * p, :],
                                             pts[i][:, 0:512], start=st, stop=sp)
                            nc.tensor.matmul(oAB[:, 512:1024], V3[kt][:, 2 * p + 1, :],
                                             pts[i][:, 512:1024], start=st, stop=sp)
                        if last:
                            # normalize prologue before the proj emits so the
                            # DVE/ACT chain starts as early as possible
                            nc.vector.tensor_copy(sc[:, :], oAB[:, :])
                            nc.scalar.activation(lnr[0:65, :], sc[0:65, :], AF.Ln)
                            with nc.allow_low_precision(reason="softmax recip"):
                                nc.scalar.activation(rr[0:65, :], lnr[0:65, :], AF.Exp,
                                                     scale=-1.0)
                        if k2 >= emit_from:
                            emit_proj()
                        if last:
                            emit_proj()

                    prev = s_step(0)
                    for k2 in range(1, KT16 // 2):
                        cur = s_step(k2)
                        if k2 == 3:
                            flush_pending(0)
                        if k2 == 4:
                            flush_pending(1)
                        o_step(k2 - 1, prev)
                        prev = cur
                    o_step(KT16 // 2 - 1, prev, last=True)

                    def _norm_a():
                        bcb = ps.tile([128, 512], F32, tag="bcb", name=f"bc{b}_{p}_{qb}a")
                        nc.tensor.matmul(bcb[:, 0:512], ones128[64:65, :], rr[64:65, 0:512],
                                         start=True, stop=True, tile_position=(64, 0))
                        with nc.allow_low_precision(reason="o norm"):
                            nc.vector.tensor_mul(OT[p][0:64, qbs], sc[0:64, 0:512], bcb[0:64, :])

                    def _norm_b():
                        bcb = ps.tile([128, 512], F32, tag="bcb", name=f"bc{b}_{p}_{qb}b")
                        nc.tensor.matmul(bcb[:, 0:512], ones128[0:1, :], rr[0:1, 512:1024],
                                         start=True, stop=True, tile_position=(0, 0))
                        with nc.allow_low_precision(reason="o norm"):
                            nc.vector.tensor_mul(OT[p][64:128, qbs], sc[64:128, 512:1024],
                                                 bcb[64:128, :])

                    pending[0] = _norm_a
                    pending[1] = _norm_b

                def unit_lone(qb):
                    """5th head: the two k-tiles of each 2-kt step share one
                    [128,1024] S tile and a single fused exp."""
                    qbs = slice(qb * QB, (qb + 1) * QB)
                    oC = ps.tile([128, 1024], F32, tag="T2", name=f"oC{b}_{qb}")
                    sc = pool.tile([128, 1024], F32, tag="sc", name=f"scl{b}_{qb}", bufs=2)
                    lnr = pool.tile([128, 1024], F32, tag="lnr", name=f"lnl{b}_{qb}", bufs=2)
                    rr = pool.tile([128, 1024], F16, tag="rr", name=f"rrl{b}_{qb}", bufs=2)

                    def s_step(k2):
                        s = ps.tile([128, 1024], F32, tag=f"T{k2 % 2}", name=f"sl{b}_{qb}_{k2}")
                        for i, kt in enumerate((2 * k2, 2 * k2 + 1)):
                            ksl = slice(kt * 128, (kt + 1) * 128)
                            nc.tensor.matmul(s[:, 512 * i:512 * i + 512],
                                             KT[2][64:128, ksl], QT[2][64:128, qbs],
                                             start=True, stop=True, tile_position=(64, 0))
                        p_t = pool.tile([128, 1024], F16, tag="p", name=f"pl{b}_{qb}_{k2}", bufs=4)
                        with nc.allow_low_precision(reason="fp16 probs"):
                            nc.scalar.activation(p_t[:], s[:], AF.Exp, scale=SCALE)
                        return p_t

                    def o_step(k2, p_t, last=False):
                        for i, kt in enumerate((2 * k2, 2 * k2 + 1)):
                            st, sp = kt == 0, kt == KT16 - 1
                            nc.tensor.matmul(oC[:, 0:512], V3[kt][:, 4, :],
                                             p_t[:, 512 * i:512 * i + 512], start=st, stop=sp)
                        if last:
                            nc.vector.tensor_copy(sc[:, 0:512], oC[:, 0:512])
                            nc.scalar.activation(lnr[0:65, 0:512], sc[0:65, 0:512], AF.Ln)
                            with nc.allow_low_precision(reason="softmax recip"):
                                nc.scalar.activation(rr[0:65, 0:512], lnr[0:65, 0:512],
                                                     AF.Exp, scale=-1.0)
                        if k2 >= 1:
                            emit_proj()
                        if last:
                            emit_proj()

                    prev = s_step(0)
                    for k2 in range(1, KT16 // 2):
                        cur = s_step(k2)
                        if k2 == 3:
                            flush_pending(0)
                        if k2 == 4:
                            flush_pending(1)
                        o_step(k2 - 1, prev)
                        prev = cur
                    o_step(KT16 // 2 - 1, prev, last=True)

                    def _norm():
                        bcb = ps.tile([128, 512], F32, tag="bcb", name=f"bcl{b}_{qb}")
                        nc.tensor.matmul(bcb[:, 0:512], ones128[64:65, :], rr[64:65, 0:512],
                                         start=True, stop=True, tile_position=(64, 0))
                        with nc.allow_low_precision(reason="o norm"):
                            nc.vector.tensor_mul(OT[2][0:64, qbs], sc[0:64, 0:512], bcb[0:64, :])

                    pending[0] = _norm

                for qb in range(NQB):
                    unit_pair(0, qb, first_after_qb=True)
                    unit_pair(1, qb, first_after_qb=False)
                    unit_lone(qb)
                    for t in range(qb * 4, qb * 4 + 4):
                        for n in range(5):
                            proj_q.append((t, n, OTb, boffb))
                nc.leave_named_scope(f"phA{b}", scope_a[0], False)

            # drain: the first two matmuls of each group don't need OT[2]
            # (written by the still-pending last normalize), so prefix them
            # onto 3 rotating banks while the normalize chain completes.
            # bcb stays reserved for the flush broadcast (reusing it for a
            # prefix would deadlock: new-tile WAR would make the broadcast
            # wait on a finish that needs the broadcast).
            groups = list(proj_q)
            proj_q.clear()
            drain_tiles = {}

            def drain_prefix(g):
                t, n, OTq, boffq = groups[g]
                tsl = slice(t * 128, (t + 1) * 128)
                nsl = slice(n * 512, (n + 1) * 512)
                yp = ps.tile([128, 512], F32, tag=["ypb", "T0", "T1"][g % 3],
                             name=f"dyp{g}")
                nc.tensor.matmul(yp[:, 0:512], OTq[0][:, tsl], wp[:, 0, nsl],
                                 start=True, stop=False)
                nc.tensor.matmul(yp[:, 0:512], OTq[1][:, tsl], wp[:, 1, nsl],
                                 start=False, stop=False)
                drain_tiles[g] = yp

            def drain_finish(g):
                t, n, OTq, boffq = groups[g]
                tsl = slice(t * 128, (t + 1) * 128)
                nsl = slice(n * 512, (n + 1) * 512)
                yp = drain_tiles.pop(g)
                nc.tensor.matmul(yp[:, 0:512], OTq[2][:, tsl], wp[:, 2, nsl],
                                 start=False, stop=True)
                yrow_put(t, n, boffq, yp)

            for g in range(min(3, len(groups))):
                drain_prefix(g)
            flush_pending(0)
            flush_pending(1)
            for g in range(len(groups)):
                drain_finish(g)
                if g + 3 < len(groups):
                    drain_prefix(g + 3)
    return nc


def kernel(x, w_qkv, b_qkv, w_proj, b_proj):
    x = np.asarray(x, np.float32)
    w_qkv = np.asarray(w_qkv, np.float32)
    b_qkv = np.asarray(b_qkv, np.float32)
    w_proj = np.asarray(w_proj, np.float32)
    b_proj = np.asarray(b_proj, np.float32)

    if "nc" not in _CACHE:
        nc = _build()
        nc.compile()
        _CACHE["nc"] = nc
    nc = _CACHE["nc"]

    # [128, chunk, K-tile, token]: each chunk loads as 128 contiguous 10KB
    # descriptor lines instead of 2560 x 512B (DMA is descriptor-rate bound)
    xH = np.ascontiguousarray(
        x.reshape(TOK // CK, CK, KC, 128).transpose(3, 0, 2, 1).astype(np.float16))
    in_maps = []
    for c in range(NCORES):
        f0 = c * HPC * D                                       # 320*c
        qcols = slice(f0, f0 + HPC * D)
        wq = w_qkv[:, qcols]
        wk = w_qkv[:, C + f0: C + f0 + HPC * D]
        wv = w_qkv[:, 2 * C + f0: 2 * C + f0 + HPC * D]
        # [q0..q3 (256) | k0..k3 (256) | q4|k4 (128) | v (320)]
        wall = np.concatenate([wq[:, 0:256], wk[:, 0:256],
                               wq[:, 256:320], wk[:, 256:320], wv], axis=1)
        wp = np.zeros((384, C), np.float16)
        wp[0:320] = w_proj[f0:f0 + HPC * D, :]
        bias = np.zeros((128, 6), np.float32)
        bq = b_qkv[qcols]
        bk = b_qkv[C + f0: C + f0 + HPC * D]
        bias[:, 0] = bq[0:128]
        bias[:, 1] = bq[128:256]
        bias[64:128, 2] = bq[256:320]
        bias[:, 3] = bk[0:128]
        bias[:, 4] = bk[128:256]
        bias[64:128, 5] = bk[256:320]
        in_maps.append({"xH": xH, "wall": np.ascontiguousarray(wall.astype(np.float16)),
                        "wp": wp, "bias": bias})

    _CACHE["in_maps"] = in_maps
    res = run_bass_kernel_spmd(nc, in_maps, core_ids=list(range(NCORES)))
    _CACHE["results"] = res.results
    y = np.zeros((TOK, C), np.float32)
    for c in range(NCORES):
        y += res.results[c]["y"].astype(np.float32)
    bias_eff = b_proj + b_qkv[2 * C:] @ w_proj                 # v-bias folded through proj
    y += bias_eff
    return y.reshape(B, N, C).astype(np.float32)
